# revision 1
# baseline (speedup 1.0000x reference)
"""MetaBaseline (retrieval_knn) Trainium2 kernel — 8-core SPMD.

Sharding: each episode's 30 queries are split over 4 cores with ranges
[0:8], [8:16], [15:23], [22:30] (ranges overlap by one query so every core
runs an identical nq=8 program; duplicated rows are dropped on gather).
Each core computes, for its queries, against its episode's full support set:
  - cosine logits (mean-pooled, PE Gram trick for norms)
  - channel-level top-5 similarity (fp16 matmuls + hw top-8 `vector.max`)
  - pixel-level top-5 similarity (dominant cost; fp16 matmuls into PSUM,
    `vector.max` straight from PSUM over 3+2 shot banks, exact hierarchical
    top-5 merge, per-query segmented sums via selector matmuls)
The tiny BatchNorm (batch stats over 30 queries) + dilated conv epilogue runs
on host from the gathered [30, 15] features.
"""
import copy
import numpy as np

import concourse.bass as bass
import concourse.mybir as mybir
from concourse.tile import TileContext
from concourse.bass_utils import run_bass_kernel_spmd

F32 = mybir.dt.float32
F16 = mybir.dt.float16

B, WAY, SHOT, K = 2, 5, 5, 5
Q_NUM, DIM, HW = 30, 64, 441
BN_EPS = 1e-5

NQ, D = 8, DIM
R = NQ * HW            # 3528 query-pixel rows per core
M = 126                # rows per chunk
CH = R // M            # 28 chunks
NS = WAY * SHOT        # 25 support maps
HC0 = [(0, 128), (128, 256), (256, 384)]
TAIL = 384
CORE_RANGES = [(0, 8), (8, 16), (15, 23), (22, 30)]


def _split_multi_waits(nc, max_waits=1):
    """walrus CTRL codegen rejects >max_waits sem-waits on one instruction;
    split extras onto preceding drains."""
    for function in nc.m.functions:
        for block in function.blocks:
            new_insts = []
            for inst in block.instructions:
                si = inst.sync_info
                if si is None or si.on_wait is None or len(si.on_wait) <= max_waits:
                    new_insts.append(inst)
                    continue
                waits = list(si.on_wait)
                extra, keep = waits[:-max_waits], waits[-max_waits:]
                ci = 0
                while extra:
                    chunk, extra = extra[:max_waits], extra[max_waits:]
                    new_insts.append(mybir.InstDrain(
                        name=f"{inst.name}-wsplit{ci}", engine=inst.engine,
                        ins=[], outs=[],
                        sync_info=mybir.SyncInfo(on_wait=chunk, on_update=[])))
                    ci += 1
                new_insts.append(copy.replace(
                    inst, sync_info=mybir.SyncInfo(
                        on_wait=keep, on_update=list(si.on_update or []))))
            block.instructions = new_insts


def _build_sel() -> np.ndarray:
    sel = np.zeros((CH, M, NQ), np.float32)
    for c in range(CH):
        for r in range(M):
            sel[c, r, (c * M + r) // HW] = 1.0
    return sel


def _build_kernel() -> bass.Bass:
    nc = bass.Bass("TRN2")
    q_d = nc.dram_tensor("q", [NQ, D, HW], F32, kind="ExternalInput")
    s_d = nc.dram_tensor("s", [NS, D, HW], F32, kind="ExternalInput")
    sel_d = nc.dram_tensor("sel", [CH, M, NQ], F32, kind="ExternalInput")
    feats_d = nc.dram_tensor("feats", [NQ, 3 * WAY], F32, kind="ExternalOutput")

    with TileContext(nc) as tc:
        with tc.tile_pool(name="big", bufs=1) as big:
            # ---------- load ----------
            q_raw = big.tile([D, R], F32)
            s_raw = big.tile([D, NS * HW], F32)
            sel_sb = big.tile([M, CH * NQ], F32)
            nc.gpsimd.dma_start(
                q_raw[:, :].rearrange("d (q h) -> d q h", h=HW),
                q_d[:, :, :].rearrange("q d h -> d q h"))
            nc.gpsimd.dma_start(
                s_raw[:, :].rearrange("d (n h) -> d n h", h=HW),
                s_d[:, :, :].rearrange("n d h -> d n h"))
            nc.gpsimd.dma_start(
                sel_sb[:, :].rearrange("r (c q) -> r c q", q=NQ),
                sel_d[:, :, :].rearrange("c r q -> r c q"))

            # ---------- stats: sumsq over h per (d, map) ----------
            sq_scr = big.tile([D, HW], F32)
            q_ss = big.tile([D, NQ], F32)
            s_ss = big.tile([D, NS], F32)
            for i in range(NQ):
                nc.scalar.activation(sq_scr[:, :], q_raw[:, i * HW:(i + 1) * HW],
                                     mybir.ActivationFunctionType.Square,
                                     accum_out=q_ss[:, i:i + 1])
            for i in range(NS):
                nc.scalar.activation(sq_scr[:, :], s_raw[:, i * HW:(i + 1) * HW],
                                     mybir.ActivationFunctionType.Square,
                                     accum_out=s_ss[:, i:i + 1])
            q_rn = big.tile([D, NQ], F32)
            s_rn = big.tile([D, NS], F32)
            nc.scalar.sqrt(q_rn[:, :], q_ss[:, :])
            nc.vector.reciprocal(q_rn[:, :], q_rn[:, :])
            nc.scalar.sqrt(s_rn[:, :], s_ss[:, :])
            nc.vector.reciprocal(s_rn[:, :], s_rn[:, :])

            # ---------- normalized descriptors (fp16) ----------
            qn = big.tile([D, R], F16)
            sn = big.tile([D, NS * HW], F16)
            for i in range(NQ):
                nc.scalar.mul(qn[:, i * HW:(i + 1) * HW],
                              q_raw[:, i * HW:(i + 1) * HW], q_rn[:, i:i + 1])
            for i in range(NS):
                nc.scalar.mul(sn[:, i * HW:(i + 1) * HW],
                              s_raw[:, i * HW:(i + 1) * HW], s_rn[:, i:i + 1])

            # zero-padded tails (h in [384, 512)) for the xbar transposes
            q_tail = big.tile([D, NQ * 128], F16)
            s_tail = big.tile([D, NS * 128], F16)
            nc.vector.memset(q_tail[:, :], 0.0)
            nc.vector.memset(s_tail[:, :], 0.0)
            for i in range(NQ):
                nc.scalar.copy(q_tail[:, i * 128:i * 128 + HW - TAIL],
                               qn[:, i * HW + TAIL:(i + 1) * HW])
            for i in range(NS):
                nc.scalar.copy(s_tail[:, i * 128:i * 128 + HW - TAIL],
                               sn[:, i * HW + TAIL:(i + 1) * HW])

            # ---------- cosine logits ----------
            q_pool = big.tile([D, NQ], F32)
            proto = big.tile([D, WAY], F32)
            mean_scr = big.tile([D, SHOT * HW], F32)
            for i in range(NQ):
                nc.scalar.activation(mean_scr[:, 0:HW], q_raw[:, i * HW:(i + 1) * HW],
                                     mybir.ActivationFunctionType.Identity,
                                     scale=1.0 / HW,
                                     accum_out=q_pool[:, i:i + 1])
            for w in range(WAY):
                nc.scalar.activation(mean_scr[:, :],
                                     s_raw[:, w * SHOT * HW:(w + 1) * SHOT * HW],
                                     mybir.ActivationFunctionType.Identity,
                                     scale=1.0 / (SHOT * HW),
                                     accum_out=proto[:, w:w + 1])
            psq = big.tile([D, NQ + WAY], F32)
            nc.scalar.square(psq[:, 0:NQ], q_pool[:, :])
            nc.scalar.square(psq[:, NQ:], proto[:, :])
            ones64 = big.tile([D, 1], F32)
            nc.vector.memset(ones64[:, :], 1.0)
            ones8 = big.tile([1, NQ], F32)
            nc.vector.memset(ones8[:, :], 1.0)
            rinv = big.tile([1, NQ + WAY], F32)
            with tc.tile_pool(name="psnrm", bufs=1, space="PSUM") as psnrm:
                pss = psnrm.tile([1, NQ + WAY], F32)
                nc.tensor.matmul(pss[:, :], ones64[:, :], psq[:, :],
                                 start=True, stop=True)
                nc.scalar.sqrt(rinv[:, :], pss[:, :])
            nc.vector.reciprocal(rinv[:, :], rinv[:, :])
            q_rinv_col = big.tile([NQ, 1], F32)
            nc.gpsimd.dma_start(q_rinv_col[:, :], rinv[0:1, 0:NQ])

            feats = big.tile([NQ, 3 * WAY], F32)

            with tc.tile_pool(name="psdot", bufs=1, space="PSUM") as psdot:
                dots = psdot.tile([NQ, WAY], F32)
                nc.tensor.matmul(dots[:, :], q_pool[:, :], proto[:, :],
                                 start=True, stop=True)
                pr_b = psdot.tile([NQ, WAY], F32)
                nc.tensor.matmul(pr_b[:, :], ones8[:, :], rinv[0:1, NQ:],
                                 start=True, stop=True)
                nc.scalar.mul(feats[:, 0:WAY], dots[:, :], q_rinv_col[:, :])
                nc.vector.tensor_mul(feats[:, 0:WAY], feats[:, 0:WAY], pr_b[:, :])

            # ---------- transposes (DMA xbar) for channel level ----------
            qn_T = big.tile([128, NQ * 4 * D], F16)
            for qi in range(NQ):
                for hc, (h0, h1) in enumerate(HC0):
                    nc.sync.dma_start_transpose(
                        qn_T[0:h1 - h0, (qi * 4 + hc) * D:(qi * 4 + hc + 1) * D],
                        qn[:, qi * HW + h0:qi * HW + h1])
                nc.sync.dma_start_transpose(
                    qn_T[0:128, (qi * 4 + 3) * D:(qi * 4 + 4) * D],
                    q_tail[:, qi * 128:(qi + 1) * 128])
            sl_T = big.tile([128, WAY * 4 * SHOT * D], F16)
            for w in range(WAY):
                for sh in range(SHOT):
                    src0 = (w * SHOT + sh) * HW
                    for hc, (h0, h1) in enumerate(HC0):
                        nc.sync.dma_start_transpose(
                            sl_T[0:h1 - h0,
                                 (w * 4 + hc) * SHOT * D + sh * D:
                                 (w * 4 + hc) * SHOT * D + (sh + 1) * D],
                            sn[:, src0 + h0:src0 + h1])
                    nc.sync.dma_start_transpose(
                        sl_T[0:128,
                             (w * 4 + 3) * SHOT * D + sh * D:
                             (w * 4 + 3) * SHOT * D + (sh + 1) * D],
                        s_tail[:, (w * SHOT + sh) * 128:(w * SHOT + sh + 1) * 128])

            # ---------- channel level ----------
            HCN = [128, 128, 128, 128]
            ch_sums = big.tile([D, NQ * WAY], F32)
            cand_ch = big.tile([D, 8], F32)
            with tc.tile_pool(name="psch", bufs=1, space="PSUM") as psch:
                for qi in range(NQ):
                    pch = psch.tile([D, WAY * 512], F32, tag="pch")
                    for hc in range(4):
                        hcn = HCN[hc]
                        for w in range(WAY):
                            nc.tensor.matmul(
                                pch[:, w * 512:w * 512 + SHOT * D],
                                qn_T[0:hcn, (qi * 4 + hc) * D:(qi * 4 + hc + 1) * D],
                                sl_T[0:hcn, (w * 4 + hc) * SHOT * D:(w * 4 + hc + 1) * SHOT * D],
                                start=(hc == 0), stop=(hc == 3))
                    for w in range(WAY):
                        nc.vector.max(out=cand_ch[:, :],
                                      in_=pch[:, w * 512:w * 512 + SHOT * D])
                        nc.vector.reduce_sum(ch_sums[:, qi * WAY + w:qi * WAY + w + 1],
                                             cand_ch[:, 0:5],
                                             axis=mybir.AxisListType.X)
            siml_sb = big.tile([1, NQ * WAY], F32)
            with tc.tile_pool(name="pssl", bufs=1, space="PSUM") as pssl:
                siml = pssl.tile([1, NQ * WAY], F32)
                nc.tensor.matmul(siml[:, :], ones64[:, :], ch_sums[:, :],
                                 start=True, stop=True)
                nc.scalar.copy(siml_sb[:, :], siml[:, :])
            nc.gpsimd.dma_start(feats[:, WAY:2 * WAY], siml_sb[0:1, :])

            # ---------- pixel level (dominant) ----------
            rs_all = big.tile([M, CH * WAY], F32)
            cand16 = big.tile([M, 16], F32)
            cand40 = big.tile([M, WAY * 8], F32)
            with tc.tile_pool(name="psA", bufs=2, space="PSUM") as psA, \
                 tc.tile_pool(name="psB", bufs=1, space="PSUM") as psB:
                for c in range(CH):
                    lhs = qn[:, c * M:(c + 1) * M]
                    for w in range(WAY):
                        A = psA.tile([M, 3 * 512], F32, tag="A")
                        B = psB.tile([M, 2 * 512], F32, tag="B")
                        for sh in range(3):
                            nc.tensor.matmul(
                                A[:, sh * 512:sh * 512 + HW], lhs,
                                sn[:, (w * SHOT + sh) * HW:(w * SHOT + sh + 1) * HW],
                                start=True, stop=True)
                        for sh in range(2):
                            nc.tensor.matmul(
                                B[:, sh * 512:sh * 512 + HW], lhs,
                                sn[:, (w * SHOT + 3 + sh) * HW:(w * SHOT + 4 + sh) * HW],
                                start=True, stop=True)
                        nc.vector.max(
                            out=cand16[:, 0:8],
                            in_=A[:, :].rearrange("m (b x) -> m b x", x=512)[:, :, :HW])
                        nc.vector.max(
                            out=cand16[:, 8:16],
                            in_=B[:, :].rearrange("m (b x) -> m b x", x=512)[:, :, :HW])
                        nc.vector.max(out=cand40[:, w * 8:(w + 1) * 8],
                                      in_=cand16[:, :])
                    nc.vector.reduce_sum(
                        rs_all[:, c * WAY:(c + 1) * WAY],
                        cand40[:, :].rearrange("m (w k) -> m w k", k=8)[:, :, 0:5],
                        axis=mybir.AxisListType.X)

            # ---------- per-query segmented sums ----------
            with tc.tile_pool(name="psP", bufs=1, space="PSUM") as psP:
                simp = psP.tile([NQ, WAY], F32)
                for c in range(CH):
                    nc.tensor.matmul(simp[:, :],
                                     sel_sb[:, c * NQ:(c + 1) * NQ],
                                     rs_all[:, c * WAY:(c + 1) * WAY],
                                     start=(c == 0), stop=(c == CH - 1))
                nc.scalar.copy(feats[:, 2 * WAY:], simp[:, :])

            nc.gpsimd.dma_start(feats_d[:, :], feats[:, :])

    _split_multi_waits(nc, max_waits=1)
    return nc


_NC = None
_SEL = None


def _get_kernel():
    global _NC, _SEL
    if _NC is None:
        _NC = _build_kernel()
        _SEL = _build_sel()
    return _NC, _SEL


def kernel(input1_batch, input2_batch, gamma, beta, conv_w):
    q = np.ascontiguousarray(np.asarray(input1_batch, dtype=np.float32)
                             .reshape(B, Q_NUM, DIM, HW))
    s = np.ascontiguousarray(np.asarray(input2_batch, dtype=np.float32)
                             .reshape(B, WAY * SHOT, DIM, HW))
    gamma = np.asarray(gamma, dtype=np.float32)
    beta = np.asarray(beta, dtype=np.float32)
    w3 = np.asarray(conv_w, dtype=np.float32).reshape(3)

    nc, sel = _get_kernel()
    in_maps = []
    for e in range(B):
        for (lo, hi) in CORE_RANGES:
            in_maps.append({
                "q": np.ascontiguousarray(q[e, lo:hi]),
                "s": s[e],
                "sel": sel,
            })
    res = run_bass_kernel_spmd(nc, in_maps, core_ids=list(range(8)))

    out = np.zeros((B, Q_NUM, WAY), np.float32)
    for e in range(B):
        feats = np.zeros((Q_NUM, 3 * WAY), np.float32)
        for ci, (lo, hi) in enumerate(CORE_RANGES):
            f = res.results[e * 4 + ci]["feats"]
            skip = 1 if ci >= 2 else 0   # drop overlapped duplicate row
            feats[lo + skip:hi] = f[skip:]
        mu = feats.mean(0)
        var = feats.var(0)
        fb = (feats - mu) / np.sqrt(var + BN_EPS) * gamma + beta
        out[e] = w3[0] * fb[:, :WAY] + w3[1] * fb[:, WAY:2 * WAY] + w3[2] * fb[:, 2 * WAY:]
    return out


# revision 3
# speedup vs baseline: 1.1476x; 1.1476x over previous
"""MetaBaseline (retrieval_knn) Trainium2 kernel — 8-core SPMD.

Sharding: each episode's 30 queries are split over 4 cores with ranges
[0:8], [8:16], [15:23], [22:30] (ranges overlap by one query so every core
runs an identical nq=8 program; duplicated rows are dropped on gather).
Each core computes, for its queries, against its episode's full support set:
  - cosine logits (mean-pooled, PE Gram trick for norms)
  - channel-level top-5 similarity (fp16 matmuls + hw top-8 `vector.max`)
  - pixel-level top-5 similarity (dominant cost; fp16 matmuls into PSUM,
    `vector.max` straight from PSUM over 3+2 shot banks, exact hierarchical
    top-5 merge, per-query segmented sums via selector matmuls)
The tiny BatchNorm (batch stats over 30 queries) + dilated conv epilogue runs
on host from the gathered [30, 15] features.

Program order is tuned so the pixel loop (DVE-bound steady state) starts as
early as possible: only the support/query stats + fp16 normalize gate it; the
cosine/channel phases are emitted after it and overlap its tail.
"""
import copy
import numpy as np

import concourse.bass as bass
import concourse.mybir as mybir
from concourse.tile import TileContext
from concourse.bass_utils import run_bass_kernel_spmd

F32 = mybir.dt.float32
F16 = mybir.dt.float16

B, WAY, SHOT, K = 2, 5, 5, 5
Q_NUM, DIM, HW = 30, 64, 441
BN_EPS = 1e-5

NQ, D = 8, DIM
R = NQ * HW            # 3528 query-pixel rows per core
M = 126                # rows per chunk
CH = R // M            # 28 chunks
NS = WAY * SHOT        # 25 support maps
HC0 = [(0, 128), (128, 256), (256, 384)]
TAIL = 384
CORE_RANGES = [(0, 8), (8, 16), (15, 23), (22, 30)]


def _split_multi_waits(nc, max_waits=1):
    """walrus CTRL codegen rejects >max_waits sem-waits on one instruction;
    split extras onto preceding drains."""
    for function in nc.m.functions:
        for block in function.blocks:
            new_insts = []
            for inst in block.instructions:
                si = inst.sync_info
                if si is None or si.on_wait is None or len(si.on_wait) <= max_waits:
                    new_insts.append(inst)
                    continue
                waits = list(si.on_wait)
                extra, keep = waits[:-max_waits], waits[-max_waits:]
                ci = 0
                while extra:
                    chunk, extra = extra[:max_waits], extra[max_waits:]
                    new_insts.append(mybir.InstDrain(
                        name=f"{inst.name}-wsplit{ci}", engine=inst.engine,
                        ins=[], outs=[],
                        sync_info=mybir.SyncInfo(on_wait=chunk, on_update=[])))
                    ci += 1
                new_insts.append(copy.replace(
                    inst, sync_info=mybir.SyncInfo(
                        on_wait=keep, on_update=list(si.on_update or []))))
            block.instructions = new_insts


def _build_sel() -> np.ndarray:
    sel = np.zeros((CH, M, NQ), np.float32)
    for c in range(CH):
        for r in range(M):
            sel[c, r, (c * M + r) // HW] = 1.0
    return sel


def _build_kernel() -> bass.Bass:
    nc = bass.Bass("TRN2")
    q_d = nc.dram_tensor("q", [NQ, D, HW], F32, kind="ExternalInput")
    s_d = nc.dram_tensor("s", [NS, D, HW], F32, kind="ExternalInput")
    sel_d = nc.dram_tensor("sel", [CH, M, NQ], F32, kind="ExternalInput")
    feats_d = nc.dram_tensor("feats", [NQ, 3 * WAY], F32, kind="ExternalOutput")

    with TileContext(nc) as tc:
        with tc.tile_pool(name="big", bufs=1) as big:
            # ---------- load ----------
            q_raw = big.tile([D, R], F32)
            s_raw = big.tile([D, NS * HW], F32)
            sel_sb = big.tile([M, CH * NQ], F32)
            nc.gpsimd.dma_start(
                s_raw[:, :].rearrange("d (n h) -> d n h", h=HW),
                s_d[:, :, :].rearrange("n d h -> d n h"))
            nc.gpsimd.dma_start(
                q_raw[:, :].rearrange("d (q h) -> d q h", h=HW),
                q_d[:, :, :].rearrange("q d h -> d q h"))
            nc.gpsimd.dma_start(
                sel_sb[:, :].rearrange("r (c q) -> r c q", q=NQ),
                sel_d[:, :, :].rearrange("c r q -> r c q"))

            # ---------- minimal prologue: stats + fp16 normalize ----------
            sq_scr = big.tile([D, HW], F32)
            q_ss = big.tile([D, NQ], F32)
            s_ss = big.tile([D, NS], F32)
            s_rn = big.tile([D, NS], F32)
            q_rn = big.tile([D, NQ], F32)
            qn = big.tile([D, R], F16)
            sn = big.tile([D, NS * HW], F16)
            for i in range(NS):
                nc.scalar.activation(sq_scr[:, :], s_raw[:, i * HW:(i + 1) * HW],
                                     mybir.ActivationFunctionType.Square,
                                     accum_out=s_ss[:, i:i + 1])
            nc.scalar.sqrt(s_rn[:, :], s_ss[:, :])
            nc.vector.reciprocal(s_rn[:, :], s_rn[:, :])
            for i in range(NS):
                nc.scalar.mul(sn[:, i * HW:(i + 1) * HW],
                              s_raw[:, i * HW:(i + 1) * HW], s_rn[:, i:i + 1])
            for i in range(NQ):
                nc.scalar.activation(sq_scr[:, :], q_raw[:, i * HW:(i + 1) * HW],
                                     mybir.ActivationFunctionType.Square,
                                     accum_out=q_ss[:, i:i + 1])
            nc.scalar.sqrt(q_rn[:, :], q_ss[:, :])
            nc.vector.reciprocal(q_rn[:, :], q_rn[:, :])
            for i in range(NQ):
                nc.scalar.mul(qn[:, i * HW:(i + 1) * HW],
                              q_raw[:, i * HW:(i + 1) * HW], q_rn[:, i:i + 1])

            # ---------- pixel level (dominant; starts as soon as qn/sn land) --
            rs_all = big.tile([M, CH * WAY], F32)
            cand16 = big.tile([M, 16], F32)
            cand40 = big.tile([M, WAY * 8], F32)
            with tc.tile_pool(name="psA", bufs=2, space="PSUM") as psA, \
                 tc.tile_pool(name="psB", bufs=1, space="PSUM") as psB:
                for c in range(CH):
                    lhs = qn[:, c * M:(c + 1) * M]
                    for w in range(WAY):
                        A = psA.tile([M, 3 * 512], F32, tag="A")
                        B = psB.tile([M, 2 * 512], F32, tag="B")
                        for sh in range(2):
                            nc.tensor.matmul(
                                B[:, sh * 512:sh * 512 + HW], lhs,
                                sn[:, (w * SHOT + 3 + sh) * HW:(w * SHOT + 4 + sh) * HW],
                                start=True, stop=True)
                        for sh in range(3):
                            nc.tensor.matmul(
                                A[:, sh * 512:sh * 512 + HW], lhs,
                                sn[:, (w * SHOT + sh) * HW:(w * SHOT + sh + 1) * HW],
                                start=True, stop=True)
                        nc.vector.max(
                            out=cand16[:, 8:16],
                            in_=B[:, :].rearrange("m (b x) -> m b x", x=512)[:, :, :HW])
                        nc.vector.max(
                            out=cand16[:, 0:8],
                            in_=A[:, :].rearrange("m (b x) -> m b x", x=512)[:, :, :HW])
                        nc.vector.max(
                            out=cand40[:, w * 8:(w + 1) * 8],
                            in_=cand16[:, :].rearrange("m (b x) -> m b x", x=8)[:, :, 0:5])
                    nc.vector.reduce_sum(
                        rs_all[:, c * WAY:(c + 1) * WAY],
                        cand40[:, :].rearrange("m (w k) -> m w k", k=8)[:, :, 0:5],
                        axis=mybir.AxisListType.X)

            # ---------- cosine logits (overlaps pixel tail) ----------
            q_pool = big.tile([D, NQ], F32)
            proto = big.tile([D, WAY], F32)
            mean_scr = big.tile([D, SHOT * HW], F32)
            for i in range(NQ):
                nc.scalar.activation(mean_scr[:, 0:HW], q_raw[:, i * HW:(i + 1) * HW],
                                     mybir.ActivationFunctionType.Identity,
                                     scale=1.0 / HW,
                                     accum_out=q_pool[:, i:i + 1])
            for w in range(WAY):
                nc.scalar.activation(mean_scr[:, :],
                                     s_raw[:, w * SHOT * HW:(w + 1) * SHOT * HW],
                                     mybir.ActivationFunctionType.Identity,
                                     scale=1.0 / (SHOT * HW),
                                     accum_out=proto[:, w:w + 1])
            psq = big.tile([D, NQ + WAY], F32)
            nc.scalar.square(psq[:, 0:NQ], q_pool[:, :])
            nc.scalar.square(psq[:, NQ:], proto[:, :])
            ones64 = big.tile([D, 1], F32)
            nc.vector.memset(ones64[:, :], 1.0)
            ones8 = big.tile([1, NQ], F32)
            nc.vector.memset(ones8[:, :], 1.0)
            rinv = big.tile([1, NQ + WAY], F32)
            feats = big.tile([NQ, 3 * WAY], F32)
            with tc.tile_pool(name="psnrm", bufs=1, space="PSUM") as psnrm:
                pss = psnrm.tile([1, NQ + WAY], F32)
                nc.tensor.matmul(pss[:, :], ones64[:, :], psq[:, :],
                                 start=True, stop=True)
                nc.scalar.sqrt(rinv[:, :], pss[:, :])
            nc.vector.reciprocal(rinv[:, :], rinv[:, :])
            q_rinv_col = big.tile([NQ, 1], F32)
            nc.gpsimd.dma_start(q_rinv_col[:, :], rinv[0:1, 0:NQ])
            with tc.tile_pool(name="psdot", bufs=1, space="PSUM") as psdot:
                dots = psdot.tile([NQ, WAY], F32)
                nc.tensor.matmul(dots[:, :], q_pool[:, :], proto[:, :],
                                 start=True, stop=True)
                pr_b = psdot.tile([NQ, WAY], F32)
                nc.tensor.matmul(pr_b[:, :], ones8[:, :], rinv[0:1, NQ:],
                                 start=True, stop=True)
                nc.scalar.mul(feats[:, 0:WAY], dots[:, :], q_rinv_col[:, :])
                nc.vector.tensor_mul(feats[:, 0:WAY], feats[:, 0:WAY], pr_b[:, :])

            # ---------- transposes (DMA xbar) for channel level ----------
            q_tail = big.tile([D, NQ * 128], F16)
            s_tail = big.tile([D, NS * 128], F16)
            nc.vector.memset(q_tail[:, :], 0.0)
            nc.vector.memset(s_tail[:, :], 0.0)
            for i in range(NQ):
                nc.scalar.copy(q_tail[:, i * 128:i * 128 + HW - TAIL],
                               qn[:, i * HW + TAIL:(i + 1) * HW])
            for i in range(NS):
                nc.scalar.copy(s_tail[:, i * 128:i * 128 + HW - TAIL],
                               sn[:, i * HW + TAIL:(i + 1) * HW])
            qn_T = big.tile([128, NQ * 4 * D], F16)
            for qi in range(NQ):
                for hc, (h0, h1) in enumerate(HC0):
                    nc.sync.dma_start_transpose(
                        qn_T[0:h1 - h0, (qi * 4 + hc) * D:(qi * 4 + hc + 1) * D],
                        qn[:, qi * HW + h0:qi * HW + h1])
                nc.sync.dma_start_transpose(
                    qn_T[0:128, (qi * 4 + 3) * D:(qi * 4 + 4) * D],
                    q_tail[:, qi * 128:(qi + 1) * 128])
            sl_T = big.tile([128, WAY * 4 * SHOT * D], F16)
            for w in range(WAY):
                for sh in range(SHOT):
                    src0 = (w * SHOT + sh) * HW
                    for hc, (h0, h1) in enumerate(HC0):
                        nc.sync.dma_start_transpose(
                            sl_T[0:h1 - h0,
                                 (w * 4 + hc) * SHOT * D + sh * D:
                                 (w * 4 + hc) * SHOT * D + (sh + 1) * D],
                            sn[:, src0 + h0:src0 + h1])
                    nc.sync.dma_start_transpose(
                        sl_T[0:128,
                             (w * 4 + 3) * SHOT * D + sh * D:
                             (w * 4 + 3) * SHOT * D + (sh + 1) * D],
                        s_tail[:, (w * SHOT + sh) * 128:(w * SHOT + sh + 1) * 128])

            # ---------- channel level (2 queries packed per PSUM tile) -------
            HCN = [128, 128, 128, 128]
            ch_sums = big.tile([128, 4 * WAY], F32)     # part: q (0-63 -> qi, 64-127 -> qi+4)
            cand_ch = big.tile([128, 8], F32)
            half_sel = big.tile([128, 2], F32)
            nc.vector.memset(half_sel[0:D, 0:1], 1.0)
            nc.vector.memset(half_sel[0:D, 1:2], 0.0)
            nc.vector.memset(half_sel[D:, 0:1], 0.0)
            nc.vector.memset(half_sel[D:, 1:2], 1.0)
            with tc.tile_pool(name="psch", bufs=1, space="PSUM") as psch:
                for pair in range(4):                   # qi = pair, qj = pair + 4
                    pch = psch.tile([128, WAY * 512], F32, tag="pch")
                    for half, qi in ((0, pair), (1, pair + 4)):
                        for hc in range(4):
                            hcn = HCN[hc]
                            for w in range(WAY):
                                nc.tensor.matmul(
                                    pch[half * D:half * D + D,
                                        w * 512:w * 512 + SHOT * D],
                                    qn_T[0:hcn, (qi * 4 + hc) * D:(qi * 4 + hc + 1) * D],
                                    sl_T[0:hcn, (w * 4 + hc) * SHOT * D:(w * 4 + hc + 1) * SHOT * D],
                                    start=(hc == 0), stop=(hc == 3))
                    for w in range(WAY):
                        nc.vector.max(out=cand_ch[:, :],
                                      in_=pch[:, w * 512:w * 512 + SHOT * D])
                        nc.vector.reduce_sum(ch_sums[:, pair * WAY + w:pair * WAY + w + 1],
                                             cand_ch[:, 0:5],
                                             axis=mybir.AxisListType.X)
            siml_sb = big.tile([2, 4 * WAY], F32)
            with tc.tile_pool(name="pssl", bufs=1, space="PSUM") as pssl:
                siml = pssl.tile([2, 4 * WAY], F32)
                nc.tensor.matmul(siml[:, :], half_sel[:, :], ch_sums[:, :],
                                 start=True, stop=True)
                nc.scalar.copy(siml_sb[:, :], siml[:, :])
            # [2(half), 4(pair) * 5(w)] -> feats rows q = half*4 + pair
            nc.gpsimd.dma_start(feats[:, WAY:2 * WAY], siml_sb[:, :])

            # ---------- per-query segmented sums ----------
            with tc.tile_pool(name="psP", bufs=1, space="PSUM") as psP:
                simp = psP.tile([NQ, WAY], F32)
                for c in range(CH):
                    nc.tensor.matmul(simp[:, :],
                                     sel_sb[:, c * NQ:(c + 1) * NQ],
                                     rs_all[:, c * WAY:(c + 1) * WAY],
                                     start=(c == 0), stop=(c == CH - 1))
                nc.scalar.copy(feats[:, 2 * WAY:], simp[:, :])

            nc.gpsimd.dma_start(feats_d[:, :], feats[:, :])

    _split_multi_waits(nc, max_waits=1)
    return nc


_NC = None
_SEL = None


def _get_kernel():
    global _NC, _SEL
    if _NC is None:
        _NC = _build_kernel()
        _SEL = _build_sel()
    return _NC, _SEL


def kernel(input1_batch, input2_batch, gamma, beta, conv_w):
    q = np.ascontiguousarray(np.asarray(input1_batch, dtype=np.float32)
                             .reshape(B, Q_NUM, DIM, HW))
    s = np.ascontiguousarray(np.asarray(input2_batch, dtype=np.float32)
                             .reshape(B, WAY * SHOT, DIM, HW))
    gamma = np.asarray(gamma, dtype=np.float32)
    beta = np.asarray(beta, dtype=np.float32)
    w3 = np.asarray(conv_w, dtype=np.float32).reshape(3)

    nc, sel = _get_kernel()
    in_maps = []
    for e in range(B):
        for (lo, hi) in CORE_RANGES:
            in_maps.append({
                "q": np.ascontiguousarray(q[e, lo:hi]),
                "s": s[e],
                "sel": sel,
            })
    res = run_bass_kernel_spmd(nc, in_maps, core_ids=list(range(8)))

    out = np.zeros((B, Q_NUM, WAY), np.float32)
    for e in range(B):
        feats = np.zeros((Q_NUM, 3 * WAY), np.float32)
        for ci, (lo, hi) in enumerate(CORE_RANGES):
            f = res.results[e * 4 + ci]["feats"]
            skip = 1 if ci >= 2 else 0   # drop overlapped duplicate row
            feats[lo + skip:hi] = f[skip:]
        mu = feats.mean(0)
        var = feats.var(0)
        fb = (feats - mu) / np.sqrt(var + BN_EPS) * gamma + beta
        out[e] = w3[0] * fb[:, :WAY] + w3[1] * fb[:, WAY:2 * WAY] + w3[2] * fb[:, 2 * WAY:]
    return out


# revision 5
# speedup vs baseline: 1.1847x; 1.0324x over previous
"""MetaBaseline (retrieval_knn) Trainium2 kernel — 8-core SPMD.

Sharding: each episode's 30 queries are split over 4 cores with ranges
[0:8], [8:16], [15:23], [22:30] (ranges overlap by one query so every core
runs an identical nq=8 program; duplicated rows are dropped on gather).
Each core computes, for its queries, against its episode's full support set:
  - cosine logits (mean-pooled, PE Gram trick for norms)
  - channel-level top-5 similarity (fp16 matmuls + hw top-8 `vector.max`)
  - pixel-level top-5 similarity (dominant cost; fp16 matmuls into PSUM,
    `vector.max` straight from PSUM over 3+2 shot banks, exact hierarchical
    top-5 merge, per-query segmented sums via selector matmuls)
The tiny BatchNorm (batch stats over 30 queries) + dilated conv epilogue runs
on host from the gathered [30, 15] features.

Program order is tuned so the pixel loop (DVE-bound steady state) starts as
early as possible: only the support/query stats + fp16 normalize gate it; the
cosine/channel phases are emitted after it and overlap its tail.
"""
import copy
import numpy as np

import concourse.bass as bass
import concourse.mybir as mybir
from concourse.tile import TileContext
from concourse.bass_utils import run_bass_kernel_spmd

F32 = mybir.dt.float32
F16 = mybir.dt.float16

B, WAY, SHOT, K = 2, 5, 5, 5
Q_NUM, DIM, HW = 30, 64, 441
BN_EPS = 1e-5

NQ, D = 8, DIM
R = NQ * HW            # 3528 query-pixel rows per core
M = 126                # rows per chunk
CH = R // M            # 28 chunks
NS = WAY * SHOT        # 25 support maps
HC0 = [(0, 128), (128, 256), (256, 384)]
TAIL = 384
CORE_RANGES = [(0, 8), (8, 16), (15, 23), (22, 30)]


def _split_multi_waits(nc, max_waits=1):
    """walrus CTRL codegen rejects >max_waits sem-waits on one instruction;
    split extras onto preceding drains."""
    for function in nc.m.functions:
        for block in function.blocks:
            new_insts = []
            for inst in block.instructions:
                si = inst.sync_info
                if si is None or si.on_wait is None or len(si.on_wait) <= max_waits:
                    new_insts.append(inst)
                    continue
                waits = list(si.on_wait)
                extra, keep = waits[:-max_waits], waits[-max_waits:]
                ci = 0
                while extra:
                    chunk, extra = extra[:max_waits], extra[max_waits:]
                    new_insts.append(mybir.InstDrain(
                        name=f"{inst.name}-wsplit{ci}", engine=inst.engine,
                        ins=[], outs=[],
                        sync_info=mybir.SyncInfo(on_wait=chunk, on_update=[])))
                    ci += 1
                new_insts.append(copy.replace(
                    inst, sync_info=mybir.SyncInfo(
                        on_wait=keep, on_update=list(si.on_update or []))))
            block.instructions = new_insts


def _build_sel() -> np.ndarray:
    sel = np.zeros((CH, M, NQ), np.float32)
    for c in range(CH):
        for r in range(M):
            sel[c, r, (c * M + r) // HW] = 1.0
    return sel


def _build_kernel() -> bass.Bass:
    nc = bass.Bass("TRN2")
    q_d = nc.dram_tensor("q", [NQ, D, HW], F32, kind="ExternalInput")
    s_d = nc.dram_tensor("s", [NS, D, HW], F32, kind="ExternalInput")
    sel_d = nc.dram_tensor("sel", [CH, M, NQ], F32, kind="ExternalInput")
    feats_d = nc.dram_tensor("feats", [NQ, 3 * WAY], F32, kind="ExternalOutput")

    with TileContext(nc) as tc:
        with tc.tile_pool(name="big", bufs=1) as big:
            # ---------- load ----------
            q_raw = big.tile([D, R], F32)
            s_raw_w = [big.tile([D, SHOT * HW], F32, name=f"sraw{w}", tag=f"sraw{w}")
                       for w in range(WAY)]
            sel_sb = big.tile([M, CH * NQ], F32)
            for w in range(WAY):
                nc.gpsimd.dma_start(
                    s_raw_w[w][:, :].rearrange("d (n h) -> d n h", h=HW),
                    s_d[w * SHOT:(w + 1) * SHOT, :, :].rearrange("n d h -> d n h"))
            nc.gpsimd.dma_start(
                q_raw[:, :].rearrange("d (q h) -> d q h", h=HW),
                q_d[:, :, :].rearrange("q d h -> d q h"))
            nc.gpsimd.dma_start(
                sel_sb[:, :].rearrange("r (c q) -> r c q", q=NQ),
                sel_d[:, :, :].rearrange("c r q -> r c q"))

            # ---------- minimal prologue: stats + fp16 normalize ----------
            sq_scr = big.tile([D, HW], F32)
            q_ss = big.tile([D, NQ], F32)
            s_ss = big.tile([D, NS], F32)
            s_rn = big.tile([D, NS], F32)
            q_rn = big.tile([D, NQ], F32)
            qn = big.tile([D, R], F16)
            sn = big.tile([D, NS * HW], F16)
            # q0 first (pixel chunk 0 needs it), then per-way support stats
            nc.scalar.activation(sq_scr[:, :], q_raw[:, 0:HW],
                                 mybir.ActivationFunctionType.Square,
                                 accum_out=q_ss[:, 0:1])
            nc.scalar.sqrt(q_rn[:, 0:1], q_ss[:, 0:1])
            nc.vector.reciprocal(q_rn[:, 0:1], q_rn[:, 0:1])
            nc.scalar.mul(qn[:, 0:HW], q_raw[:, 0:HW], q_rn[:, 0:1])
            for w in range(WAY):
                for sh in range(SHOT):
                    i = w * SHOT + sh
                    nc.scalar.activation(sq_scr[:, :],
                                         s_raw_w[w][:, sh * HW:(sh + 1) * HW],
                                         mybir.ActivationFunctionType.Square,
                                         accum_out=s_ss[:, i:i + 1])
                nc.scalar.sqrt(s_rn[:, w * SHOT:(w + 1) * SHOT],
                               s_ss[:, w * SHOT:(w + 1) * SHOT])
                nc.vector.reciprocal(s_rn[:, w * SHOT:(w + 1) * SHOT],
                                     s_rn[:, w * SHOT:(w + 1) * SHOT])
                for sh in range(SHOT):
                    i = w * SHOT + sh
                    nc.scalar.mul(sn[:, i * HW:(i + 1) * HW],
                                  s_raw_w[w][:, sh * HW:(sh + 1) * HW],
                                  s_rn[:, i:i + 1])
            for i in range(1, NQ):
                nc.scalar.activation(sq_scr[:, :], q_raw[:, i * HW:(i + 1) * HW],
                                     mybir.ActivationFunctionType.Square,
                                     accum_out=q_ss[:, i:i + 1])
            nc.scalar.sqrt(q_rn[:, 1:], q_ss[:, 1:])
            nc.vector.reciprocal(q_rn[:, 1:], q_rn[:, 1:])
            for i in range(1, NQ):
                nc.scalar.mul(qn[:, i * HW:(i + 1) * HW],
                              q_raw[:, i * HW:(i + 1) * HW], q_rn[:, i:i + 1])

            # ---------- pixel level (dominant; starts as soon as qn/sn land) --
            rs_all = big.tile([M, CH * WAY], F32)
            cand16 = big.tile([M, 16], F32)
            cand40 = big.tile([M, WAY * 8], F32)
            with tc.tile_pool(name="psA", bufs=2, space="PSUM") as psA, \
                 tc.tile_pool(name="psB", bufs=1, space="PSUM") as psB:
                for c in range(CH):
                    lhs = qn[:, c * M:(c + 1) * M]
                    for w in range(WAY):
                        A = psA.tile([M, 3 * 512], F32, tag="A")
                        B = psB.tile([M, 2 * 512], F32, tag="B")
                        for sh in range(2):
                            nc.tensor.matmul(
                                B[:, sh * 512:sh * 512 + HW], lhs,
                                sn[:, (w * SHOT + 3 + sh) * HW:(w * SHOT + 4 + sh) * HW],
                                start=True, stop=True)
                        for sh in range(3):
                            nc.tensor.matmul(
                                A[:, sh * 512:sh * 512 + HW], lhs,
                                sn[:, (w * SHOT + sh) * HW:(w * SHOT + sh + 1) * HW],
                                start=True, stop=True)
                        nc.vector.max(
                            out=cand16[:, 8:16],
                            in_=B[:, :].rearrange("m (b x) -> m b x", x=512)[:, :, :HW])
                        nc.vector.max(
                            out=cand16[:, 0:8],
                            in_=A[:, :].rearrange("m (b x) -> m b x", x=512)[:, :, :HW])
                        nc.vector.max(
                            out=cand40[:, w * 8:(w + 1) * 8],
                            in_=cand16[:, :].rearrange("m (b x) -> m b x", x=8)[:, :, 0:5])
                    nc.vector.reduce_sum(
                        rs_all[:, c * WAY:(c + 1) * WAY],
                        cand40[:, :].rearrange("m (w k) -> m w k", k=8)[:, :, 0:5],
                        axis=mybir.AxisListType.X)

            # ---------- cosine logits (overlaps pixel tail) ----------
            q_pool = big.tile([D, NQ], F32)
            proto = big.tile([D, WAY], F32)
            mean_scr = big.tile([D, SHOT * HW], F32)
            for i in range(NQ):
                nc.scalar.activation(mean_scr[:, 0:HW], q_raw[:, i * HW:(i + 1) * HW],
                                     mybir.ActivationFunctionType.Identity,
                                     scale=1.0 / HW,
                                     accum_out=q_pool[:, i:i + 1])
            for w in range(WAY):
                nc.scalar.activation(mean_scr[:, :], s_raw_w[w][:, :],
                                     mybir.ActivationFunctionType.Identity,
                                     scale=1.0 / (SHOT * HW),
                                     accum_out=proto[:, w:w + 1])
            psq = big.tile([D, NQ + WAY], F32)
            nc.scalar.square(psq[:, 0:NQ], q_pool[:, :])
            nc.scalar.square(psq[:, NQ:], proto[:, :])
            ones64 = big.tile([D, 1], F32)
            nc.vector.memset(ones64[:, :], 1.0)
            ones8 = big.tile([1, NQ], F32)
            nc.vector.memset(ones8[:, :], 1.0)
            rinv = big.tile([1, NQ + WAY], F32)
            feats = big.tile([NQ, 3 * WAY], F32)
            with tc.tile_pool(name="psnrm", bufs=1, space="PSUM") as psnrm:
                pss = psnrm.tile([1, NQ + WAY], F32)
                nc.tensor.matmul(pss[:, :], ones64[:, :], psq[:, :],
                                 start=True, stop=True)
                nc.scalar.sqrt(rinv[:, :], pss[:, :])
            nc.vector.reciprocal(rinv[:, :], rinv[:, :])
            q_rinv_col = big.tile([NQ, 1], F32)
            nc.gpsimd.dma_start(q_rinv_col[:, :], rinv[0:1, 0:NQ])
            with tc.tile_pool(name="psdot", bufs=1, space="PSUM") as psdot:
                dots = psdot.tile([NQ, WAY], F32)
                nc.tensor.matmul(dots[:, :], q_pool[:, :], proto[:, :],
                                 start=True, stop=True)
                pr_b = psdot.tile([NQ, WAY], F32)
                nc.tensor.matmul(pr_b[:, :], ones8[:, :], rinv[0:1, NQ:],
                                 start=True, stop=True)
                nc.scalar.mul(feats[:, 0:WAY], dots[:, :], q_rinv_col[:, :])
                nc.vector.tensor_mul(feats[:, 0:WAY], feats[:, 0:WAY], pr_b[:, :])

            # ---------- transposes (DMA xbar) for channel level ----------
            q_tail = big.tile([D, NQ * 128], F16)
            s_tail = big.tile([D, NS * 128], F16)
            nc.vector.memset(q_tail[:, :], 0.0)
            nc.vector.memset(s_tail[:, :], 0.0)
            for i in range(NQ):
                nc.scalar.copy(q_tail[:, i * 128:i * 128 + HW - TAIL],
                               qn[:, i * HW + TAIL:(i + 1) * HW])
            for i in range(NS):
                nc.scalar.copy(s_tail[:, i * 128:i * 128 + HW - TAIL],
                               sn[:, i * HW + TAIL:(i + 1) * HW])
            qn_T = big.tile([128, NQ * 4 * D], F16)
            for qi in range(NQ):
                for hc, (h0, h1) in enumerate(HC0):
                    nc.sync.dma_start_transpose(
                        qn_T[0:h1 - h0, (qi * 4 + hc) * D:(qi * 4 + hc + 1) * D],
                        qn[:, qi * HW + h0:qi * HW + h1])
                nc.sync.dma_start_transpose(
                    qn_T[0:128, (qi * 4 + 3) * D:(qi * 4 + 4) * D],
                    q_tail[:, qi * 128:(qi + 1) * 128])
            sl_T = big.tile([128, WAY * 4 * SHOT * D], F16)
            for w in range(WAY):
                for sh in range(SHOT):
                    src0 = (w * SHOT + sh) * HW
                    for hc, (h0, h1) in enumerate(HC0):
                        nc.sync.dma_start_transpose(
                            sl_T[0:h1 - h0,
                                 (w * 4 + hc) * SHOT * D + sh * D:
                                 (w * 4 + hc) * SHOT * D + (sh + 1) * D],
                            sn[:, src0 + h0:src0 + h1])
                    nc.sync.dma_start_transpose(
                        sl_T[0:128,
                             (w * 4 + 3) * SHOT * D + sh * D:
                             (w * 4 + 3) * SHOT * D + (sh + 1) * D],
                        s_tail[:, (w * SHOT + sh) * 128:(w * SHOT + sh + 1) * 128])

            # ---------- channel level (2 queries packed per PSUM tile) -------
            HCN = [128, 128, 128, 128]
            ch_sums = big.tile([128, 4 * WAY], F32)     # part: q (0-63 -> qi, 64-127 -> qi+4)
            cand_ch = big.tile([128, 8], F32)
            half_sel = big.tile([128, 2], F32)
            nc.vector.memset(half_sel[0:D, 0:1], 1.0)
            nc.vector.memset(half_sel[0:D, 1:2], 0.0)
            nc.vector.memset(half_sel[D:, 0:1], 0.0)
            nc.vector.memset(half_sel[D:, 1:2], 1.0)
            with tc.tile_pool(name="psch", bufs=1, space="PSUM") as psch:
                for pair in range(4):                   # qi = pair, qj = pair + 4
                    pch = psch.tile([128, WAY * 512], F32, tag="pch")
                    for half, qi in ((0, pair), (1, pair + 4)):
                        for hc in range(4):
                            hcn = HCN[hc]
                            for w in range(WAY):
                                nc.tensor.matmul(
                                    pch[half * D:half * D + D,
                                        w * 512:w * 512 + SHOT * D],
                                    qn_T[0:hcn, (qi * 4 + hc) * D:(qi * 4 + hc + 1) * D],
                                    sl_T[0:hcn, (w * 4 + hc) * SHOT * D:(w * 4 + hc + 1) * SHOT * D],
                                    start=(hc == 0), stop=(hc == 3))
                    for w in range(WAY):
                        nc.vector.max(out=cand_ch[:, :],
                                      in_=pch[:, w * 512:w * 512 + SHOT * D])
                        nc.vector.reduce_sum(ch_sums[:, pair * WAY + w:pair * WAY + w + 1],
                                             cand_ch[:, 0:5],
                                             axis=mybir.AxisListType.X)
            siml_sb = big.tile([2, 4 * WAY], F32)
            with tc.tile_pool(name="pssl", bufs=1, space="PSUM") as pssl:
                siml = pssl.tile([2, 4 * WAY], F32)
                nc.tensor.matmul(siml[:, :], half_sel[:, :], ch_sums[:, :],
                                 start=True, stop=True)
                nc.scalar.copy(siml_sb[:, :], siml[:, :])
            # [2(half), 4(pair) * 5(w)] -> feats rows q = half*4 + pair
            nc.gpsimd.dma_start(feats[:, WAY:2 * WAY], siml_sb[:, :])

            # ---------- per-query segmented sums ----------
            with tc.tile_pool(name="psP", bufs=1, space="PSUM") as psP:
                simp = psP.tile([NQ, WAY], F32)
                for c in range(CH):
                    nc.tensor.matmul(simp[:, :],
                                     sel_sb[:, c * NQ:(c + 1) * NQ],
                                     rs_all[:, c * WAY:(c + 1) * WAY],
                                     start=(c == 0), stop=(c == CH - 1))
                nc.scalar.copy(feats[:, 2 * WAY:], simp[:, :])

            nc.gpsimd.dma_start(feats_d[:, :], feats[:, :])

    _split_multi_waits(nc, max_waits=1)
    return nc


_NC = None
_SEL = None


def _get_kernel():
    global _NC, _SEL
    if _NC is None:
        _NC = _build_kernel()
        _SEL = _build_sel()
    return _NC, _SEL


def kernel(input1_batch, input2_batch, gamma, beta, conv_w):
    q = np.ascontiguousarray(np.asarray(input1_batch, dtype=np.float32)
                             .reshape(B, Q_NUM, DIM, HW))
    s = np.ascontiguousarray(np.asarray(input2_batch, dtype=np.float32)
                             .reshape(B, WAY * SHOT, DIM, HW))
    gamma = np.asarray(gamma, dtype=np.float32)
    beta = np.asarray(beta, dtype=np.float32)
    w3 = np.asarray(conv_w, dtype=np.float32).reshape(3)

    nc, sel = _get_kernel()
    in_maps = []
    for e in range(B):
        for (lo, hi) in CORE_RANGES:
            in_maps.append({
                "q": np.ascontiguousarray(q[e, lo:hi]),
                "s": s[e],
                "sel": sel,
            })
    res = run_bass_kernel_spmd(nc, in_maps, core_ids=list(range(8)))

    out = np.zeros((B, Q_NUM, WAY), np.float32)
    for e in range(B):
        feats = np.zeros((Q_NUM, 3 * WAY), np.float32)
        for ci, (lo, hi) in enumerate(CORE_RANGES):
            f = res.results[e * 4 + ci]["feats"]
            skip = 1 if ci >= 2 else 0   # drop overlapped duplicate row
            feats[lo + skip:hi] = f[skip:]
        mu = feats.mean(0)
        var = feats.var(0)
        fb = (feats - mu) / np.sqrt(var + BN_EPS) * gamma + beta
        out[e] = w3[0] * fb[:, :WAY] + w3[1] * fb[:, WAY:2 * WAY] + w3[2] * fb[:, 2 * WAY:]
    return out


# revision 6
# speedup vs baseline: 1.2929x; 1.0913x over previous
"""MetaBaseline (retrieval_knn) Trainium2 kernel — 8-core SPMD.

Sharding: each episode's 30 queries are split over 4 cores with ranges
[0:8], [8:16], [15:23], [22:30] (ranges overlap by one query so every core
runs an identical nq=8 program; duplicated rows are dropped on gather).
Each core computes, for its queries, against its episode's full support set:
  - cosine logits (mean-pooled, PE Gram trick for norms)
  - channel-level top-5 similarity (fp16 matmuls + hw top-8 `vector.max`)
  - pixel-level top-5 similarity (dominant cost; fp16 matmuls into PSUM,
    `vector.max` straight from PSUM over 3+2 shot banks, exact hierarchical
    top-5 merge, per-query segmented sums via selector matmuls)
The tiny BatchNorm (batch stats over 30 queries) + dilated conv epilogue runs
on host from the gathered [30, 15] features.

Program order is tuned so the pixel loop (DVE-bound steady state) starts as
early as possible: only the support/query stats + fp16 normalize gate it; the
cosine/channel phases are emitted after it and overlap its tail.
"""
import copy
import numpy as np

import concourse.bass as bass
import concourse.mybir as mybir
from concourse.tile import TileContext
from concourse.bass_utils import run_bass_kernel_spmd

F32 = mybir.dt.float32
F16 = mybir.dt.float16
STAGE_DT = mybir.dt.float32

B, WAY, SHOT, K = 2, 5, 5, 5
Q_NUM, DIM, HW = 30, 64, 441
BN_EPS = 1e-5

NQ, D = 8, DIM
R = NQ * HW            # 3528 query-pixel rows per core
M = 126                # rows per chunk
CH = R // M            # 28 chunks
NS = WAY * SHOT        # 25 support maps
HC0 = [(0, 128), (128, 256), (256, 384)]
TAIL = 384
CORE_RANGES = [(0, 8), (8, 16), (15, 23), (22, 30)]


def _split_multi_waits(nc, max_waits=1):
    """walrus CTRL codegen rejects >max_waits sem-waits on one instruction;
    split extras onto preceding drains."""
    for function in nc.m.functions:
        for block in function.blocks:
            new_insts = []
            for inst in block.instructions:
                si = inst.sync_info
                if si is None or si.on_wait is None or len(si.on_wait) <= max_waits:
                    new_insts.append(inst)
                    continue
                waits = list(si.on_wait)
                extra, keep = waits[:-max_waits], waits[-max_waits:]
                ci = 0
                while extra:
                    chunk, extra = extra[:max_waits], extra[max_waits:]
                    new_insts.append(mybir.InstDrain(
                        name=f"{inst.name}-wsplit{ci}", engine=inst.engine,
                        ins=[], outs=[],
                        sync_info=mybir.SyncInfo(on_wait=chunk, on_update=[])))
                    ci += 1
                new_insts.append(copy.replace(
                    inst, sync_info=mybir.SyncInfo(
                        on_wait=keep, on_update=list(si.on_update or []))))
            block.instructions = new_insts


def _build_sel() -> np.ndarray:
    sel = np.zeros((CH, M, NQ), np.float32)
    for c in range(CH):
        for r in range(M):
            sel[c, r, (c * M + r) // HW] = 1.0
    return sel


def _build_kernel() -> bass.Bass:
    nc = bass.Bass("TRN2")
    q_d = nc.dram_tensor("q", [NQ, D, HW], F32, kind="ExternalInput")
    s_d = nc.dram_tensor("s", [NS, D, HW], F32, kind="ExternalInput")
    sel_d = nc.dram_tensor("sel", [CH, M, NQ], F32, kind="ExternalInput")
    feats_d = nc.dram_tensor("feats", [NQ, 3 * WAY], F32, kind="ExternalOutput")

    with TileContext(nc) as tc:
        with tc.tile_pool(name="big", bufs=1) as big:
            # ---------- load ----------
            q_raw = big.tile([D, R], F32)
            s_raw_w = [big.tile([D, SHOT * HW], F32, name=f"sraw{w}", tag=f"sraw{w}")
                       for w in range(WAY)]
            sel_sb = big.tile([M, CH * NQ], F32)
            for w in range(WAY):
                nc.gpsimd.dma_start(
                    s_raw_w[w][:, :].rearrange("d (n h) -> d n h", h=HW),
                    s_d[w * SHOT:(w + 1) * SHOT, :, :].rearrange("n d h -> d n h"))
            nc.gpsimd.dma_start(
                q_raw[:, :].rearrange("d (q h) -> d q h", h=HW),
                q_d[:, :, :].rearrange("q d h -> d q h"))
            nc.gpsimd.dma_start(
                sel_sb[:, :].rearrange("r (c q) -> r c q", q=NQ),
                sel_d[:, :, :].rearrange("c r q -> r c q"))

            # ---------- minimal prologue: stats + fp16 normalize ----------
            sq_scr = big.tile([D, HW], F32)
            q_ss = big.tile([D, NQ], F32)
            s_ss = big.tile([D, NS], F32)
            s_rn = big.tile([D, NS], F32)
            q_rn = big.tile([D, NQ], F32)
            qn = big.tile([D, R], F16)
            sn = big.tile([D, NS * HW], F16)
            # q0 first (pixel chunk 0 needs it), then per-way support stats
            nc.scalar.activation(sq_scr[:, :], q_raw[:, 0:HW],
                                 mybir.ActivationFunctionType.Square,
                                 accum_out=q_ss[:, 0:1])
            nc.scalar.sqrt(q_rn[:, 0:1], q_ss[:, 0:1])
            nc.vector.reciprocal(q_rn[:, 0:1], q_rn[:, 0:1])
            nc.scalar.mul(qn[:, 0:HW], q_raw[:, 0:HW], q_rn[:, 0:1])
            for w in range(WAY):
                for sh in range(SHOT):
                    i = w * SHOT + sh
                    nc.scalar.activation(sq_scr[:, :],
                                         s_raw_w[w][:, sh * HW:(sh + 1) * HW],
                                         mybir.ActivationFunctionType.Square,
                                         accum_out=s_ss[:, i:i + 1])
                nc.scalar.sqrt(s_rn[:, w * SHOT:(w + 1) * SHOT],
                               s_ss[:, w * SHOT:(w + 1) * SHOT])
                nc.vector.reciprocal(s_rn[:, w * SHOT:(w + 1) * SHOT],
                                     s_rn[:, w * SHOT:(w + 1) * SHOT])
                for sh in range(SHOT):
                    i = w * SHOT + sh
                    nc.scalar.mul(sn[:, i * HW:(i + 1) * HW],
                                  s_raw_w[w][:, sh * HW:(sh + 1) * HW],
                                  s_rn[:, i:i + 1])
            for i in range(1, NQ):
                nc.scalar.activation(sq_scr[:, :], q_raw[:, i * HW:(i + 1) * HW],
                                     mybir.ActivationFunctionType.Square,
                                     accum_out=q_ss[:, i:i + 1])
            nc.scalar.sqrt(q_rn[:, 1:], q_ss[:, 1:])
            nc.vector.reciprocal(q_rn[:, 1:], q_rn[:, 1:])
            for i in range(1, NQ):
                nc.scalar.mul(qn[:, i * HW:(i + 1) * HW],
                              q_raw[:, i * HW:(i + 1) * HW], q_rn[:, i:i + 1])

            # ---------- pixel level (dominant; starts as soon as qn/sn land) --
            rs_all = big.tile([M, CH * WAY], F32)
            cand40 = big.tile([M, WAY * 8], F32)
            with tc.tile_pool(name="stg", bufs=3) as stg, \
                 tc.tile_pool(name="psA", bufs=2, space="PSUM") as psA, \
                 tc.tile_pool(name="psB", bufs=1, space="PSUM") as psB:
                for c in range(CH):
                    lhs = qn[:, c * M:(c + 1) * M]
                    for w in range(WAY):
                        A = psA.tile([M, 3 * 512], F32, tag="A")
                        B = psB.tile([M, 2 * 512], F32, tag="B")
                        stage = stg.tile([M, SHOT * HW], STAGE_DT, tag="stage")
                        for sh in range(2):
                            nc.tensor.matmul(
                                B[:, sh * 512:sh * 512 + HW], lhs,
                                sn[:, (w * SHOT + 3 + sh) * HW:(w * SHOT + 4 + sh) * HW],
                                start=True, stop=True)
                        for sh in range(3):
                            nc.tensor.matmul(
                                A[:, sh * 512:sh * 512 + HW], lhs,
                                sn[:, (w * SHOT + sh) * HW:(w * SHOT + sh + 1) * HW],
                                start=True, stop=True)
                        nc.scalar.copy(
                            stage[:, 3 * HW:5 * HW],
                            B[:, :].rearrange("m (b x) -> m b x", x=512)[:, :, :HW])
                        nc.scalar.copy(
                            stage[:, 0:3 * HW],
                            A[:, :].rearrange("m (b x) -> m b x", x=512)[:, :, :HW])
                        nc.vector.max(out=cand40[:, w * 8:(w + 1) * 8],
                                      in_=stage[:, :])
                    nc.vector.reduce_sum(
                        rs_all[:, c * WAY:(c + 1) * WAY],
                        cand40[:, :].rearrange("m (w k) -> m w k", k=8)[:, :, 0:5],
                        axis=mybir.AxisListType.X)

            # ---------- cosine logits (overlaps pixel tail) ----------
            q_pool = big.tile([D, NQ], F32)
            proto = big.tile([D, WAY], F32)
            mean_scr = big.tile([D, SHOT * HW], F32)
            for i in range(NQ):
                nc.scalar.activation(mean_scr[:, 0:HW], q_raw[:, i * HW:(i + 1) * HW],
                                     mybir.ActivationFunctionType.Identity,
                                     scale=1.0 / HW,
                                     accum_out=q_pool[:, i:i + 1])
            for w in range(WAY):
                nc.scalar.activation(mean_scr[:, :], s_raw_w[w][:, :],
                                     mybir.ActivationFunctionType.Identity,
                                     scale=1.0 / (SHOT * HW),
                                     accum_out=proto[:, w:w + 1])
            psq = big.tile([D, NQ + WAY], F32)
            nc.scalar.square(psq[:, 0:NQ], q_pool[:, :])
            nc.scalar.square(psq[:, NQ:], proto[:, :])
            ones64 = big.tile([D, 1], F32)
            nc.vector.memset(ones64[:, :], 1.0)
            ones8 = big.tile([1, NQ], F32)
            nc.vector.memset(ones8[:, :], 1.0)
            rinv = big.tile([1, NQ + WAY], F32)
            feats = big.tile([NQ, 3 * WAY], F32)
            with tc.tile_pool(name="psnrm", bufs=1, space="PSUM") as psnrm:
                pss = psnrm.tile([1, NQ + WAY], F32)
                nc.tensor.matmul(pss[:, :], ones64[:, :], psq[:, :],
                                 start=True, stop=True)
                nc.scalar.sqrt(rinv[:, :], pss[:, :])
            nc.vector.reciprocal(rinv[:, :], rinv[:, :])
            q_rinv_col = big.tile([NQ, 1], F32)
            nc.gpsimd.dma_start(q_rinv_col[:, :], rinv[0:1, 0:NQ])
            with tc.tile_pool(name="psdot", bufs=1, space="PSUM") as psdot:
                dots = psdot.tile([NQ, WAY], F32)
                nc.tensor.matmul(dots[:, :], q_pool[:, :], proto[:, :],
                                 start=True, stop=True)
                pr_b = psdot.tile([NQ, WAY], F32)
                nc.tensor.matmul(pr_b[:, :], ones8[:, :], rinv[0:1, NQ:],
                                 start=True, stop=True)
                nc.scalar.mul(feats[:, 0:WAY], dots[:, :], q_rinv_col[:, :])
                nc.vector.tensor_mul(feats[:, 0:WAY], feats[:, 0:WAY], pr_b[:, :])

            # ---------- transposes (DMA xbar) for channel level ----------
            q_tail = big.tile([D, NQ * 128], F16)
            s_tail = big.tile([D, NS * 128], F16)
            nc.vector.memset(q_tail[:, :], 0.0)
            nc.vector.memset(s_tail[:, :], 0.0)
            for i in range(NQ):
                nc.scalar.copy(q_tail[:, i * 128:i * 128 + HW - TAIL],
                               qn[:, i * HW + TAIL:(i + 1) * HW])
            for i in range(NS):
                nc.scalar.copy(s_tail[:, i * 128:i * 128 + HW - TAIL],
                               sn[:, i * HW + TAIL:(i + 1) * HW])
            qn_T = big.tile([128, NQ * 4 * D], F16)
            for qi in range(NQ):
                for hc, (h0, h1) in enumerate(HC0):
                    nc.sync.dma_start_transpose(
                        qn_T[0:h1 - h0, (qi * 4 + hc) * D:(qi * 4 + hc + 1) * D],
                        qn[:, qi * HW + h0:qi * HW + h1])
                nc.sync.dma_start_transpose(
                    qn_T[0:128, (qi * 4 + 3) * D:(qi * 4 + 4) * D],
                    q_tail[:, qi * 128:(qi + 1) * 128])
            sl_T = big.tile([128, WAY * 4 * SHOT * D], F16)
            for w in range(WAY):
                for sh in range(SHOT):
                    src0 = (w * SHOT + sh) * HW
                    for hc, (h0, h1) in enumerate(HC0):
                        nc.sync.dma_start_transpose(
                            sl_T[0:h1 - h0,
                                 (w * 4 + hc) * SHOT * D + sh * D:
                                 (w * 4 + hc) * SHOT * D + (sh + 1) * D],
                            sn[:, src0 + h0:src0 + h1])
                    nc.sync.dma_start_transpose(
                        sl_T[0:128,
                             (w * 4 + 3) * SHOT * D + sh * D:
                             (w * 4 + 3) * SHOT * D + (sh + 1) * D],
                        s_tail[:, (w * SHOT + sh) * 128:(w * SHOT + sh + 1) * 128])

            # ---------- channel level (2 queries packed per PSUM tile) -------
            HCN = [128, 128, 128, 128]
            ch_sums = big.tile([128, 4 * WAY], F32)     # part: q (0-63 -> qi, 64-127 -> qi+4)
            cand_ch = big.tile([128, 8], F32)
            half_sel = big.tile([128, 2], F32)
            nc.vector.memset(half_sel[0:D, 0:1], 1.0)
            nc.vector.memset(half_sel[0:D, 1:2], 0.0)
            nc.vector.memset(half_sel[D:, 0:1], 0.0)
            nc.vector.memset(half_sel[D:, 1:2], 1.0)
            with tc.tile_pool(name="psch", bufs=1, space="PSUM") as psch:
                for pair in range(4):                   # qi = pair, qj = pair + 4
                    pch = psch.tile([128, WAY * 512], F32, tag="pch")
                    for half, qi in ((0, pair), (1, pair + 4)):
                        for hc in range(4):
                            hcn = HCN[hc]
                            for w in range(WAY):
                                nc.tensor.matmul(
                                    pch[half * D:half * D + D,
                                        w * 512:w * 512 + SHOT * D],
                                    qn_T[0:hcn, (qi * 4 + hc) * D:(qi * 4 + hc + 1) * D],
                                    sl_T[0:hcn, (w * 4 + hc) * SHOT * D:(w * 4 + hc + 1) * SHOT * D],
                                    start=(hc == 0), stop=(hc == 3))
                    for w in range(WAY):
                        nc.vector.max(out=cand_ch[:, :],
                                      in_=pch[:, w * 512:w * 512 + SHOT * D])
                        nc.vector.reduce_sum(ch_sums[:, pair * WAY + w:pair * WAY + w + 1],
                                             cand_ch[:, 0:5],
                                             axis=mybir.AxisListType.X)
            siml_sb = big.tile([2, 4 * WAY], F32)
            with tc.tile_pool(name="pssl", bufs=1, space="PSUM") as pssl:
                siml = pssl.tile([2, 4 * WAY], F32)
                nc.tensor.matmul(siml[:, :], half_sel[:, :], ch_sums[:, :],
                                 start=True, stop=True)
                nc.scalar.copy(siml_sb[:, :], siml[:, :])
            # [2(half), 4(pair) * 5(w)] -> feats rows q = half*4 + pair
            nc.gpsimd.dma_start(feats[:, WAY:2 * WAY], siml_sb[:, :])

            # ---------- per-query segmented sums ----------
            with tc.tile_pool(name="psP", bufs=1, space="PSUM") as psP:
                simp = psP.tile([NQ, WAY], F32)
                for c in range(CH):
                    nc.tensor.matmul(simp[:, :],
                                     sel_sb[:, c * NQ:(c + 1) * NQ],
                                     rs_all[:, c * WAY:(c + 1) * WAY],
                                     start=(c == 0), stop=(c == CH - 1))
                nc.scalar.copy(feats[:, 2 * WAY:], simp[:, :])

            nc.gpsimd.dma_start(feats_d[:, :], feats[:, :])

    _split_multi_waits(nc, max_waits=1)
    return nc


_NC = None
_SEL = None


def _get_kernel():
    global _NC, _SEL
    if _NC is None:
        _NC = _build_kernel()
        _SEL = _build_sel()
    return _NC, _SEL


def kernel(input1_batch, input2_batch, gamma, beta, conv_w):
    q = np.ascontiguousarray(np.asarray(input1_batch, dtype=np.float32)
                             .reshape(B, Q_NUM, DIM, HW))
    s = np.ascontiguousarray(np.asarray(input2_batch, dtype=np.float32)
                             .reshape(B, WAY * SHOT, DIM, HW))
    gamma = np.asarray(gamma, dtype=np.float32)
    beta = np.asarray(beta, dtype=np.float32)
    w3 = np.asarray(conv_w, dtype=np.float32).reshape(3)

    nc, sel = _get_kernel()
    in_maps = []
    for e in range(B):
        for (lo, hi) in CORE_RANGES:
            in_maps.append({
                "q": np.ascontiguousarray(q[e, lo:hi]),
                "s": s[e],
                "sel": sel,
            })
    res = run_bass_kernel_spmd(nc, in_maps, core_ids=list(range(8)))

    out = np.zeros((B, Q_NUM, WAY), np.float32)
    for e in range(B):
        feats = np.zeros((Q_NUM, 3 * WAY), np.float32)
        for ci, (lo, hi) in enumerate(CORE_RANGES):
            f = res.results[e * 4 + ci]["feats"]
            skip = 1 if ci >= 2 else 0   # drop overlapped duplicate row
            feats[lo + skip:hi] = f[skip:]
        mu = feats.mean(0)
        var = feats.var(0)
        fb = (feats - mu) / np.sqrt(var + BN_EPS) * gamma + beta
        out[e] = w3[0] * fb[:, :WAY] + w3[1] * fb[:, WAY:2 * WAY] + w3[2] * fb[:, 2 * WAY:]
    return out


# revision 9
# speedup vs baseline: 1.3166x; 1.0183x over previous
"""MetaBaseline (retrieval_knn) Trainium2 kernel — 8-core SPMD.

Sharding: each episode's 30 queries are split over 4 cores with ranges
[0:8], [8:16], [15:23], [22:30] (ranges overlap by one query so every core
runs an identical nq=8 program; duplicated rows are dropped on gather).
Each core computes, for its queries, against its episode's full support set:
  - cosine logits (mean-pooled, PE Gram trick for norms)
  - channel-level top-5 similarity (fp16 matmuls + hw top-8 `vector.max`)
  - pixel-level top-5 similarity (dominant cost; fp16 matmuls into PSUM,
    `vector.max` straight from PSUM over 3+2 shot banks, exact hierarchical
    top-5 merge, per-query segmented sums via selector matmuls)
The tiny BatchNorm (batch stats over 30 queries) + dilated conv epilogue runs
on host from the gathered [30, 15] features.

Program order is tuned so the pixel loop (DVE-bound steady state) starts as
early as possible: only the support/query stats + fp16 normalize gate it; the
cosine/channel phases are emitted after it and overlap its tail.
"""
import copy
import numpy as np

import concourse.bass as bass
import concourse.mybir as mybir
from concourse.tile import TileContext
from concourse.bass_utils import run_bass_kernel_spmd

F32 = mybir.dt.float32
F16 = mybir.dt.float16
STAGE_DT = mybir.dt.float32
STG_BUFS = 6

B, WAY, SHOT, K = 2, 5, 5, 5
Q_NUM, DIM, HW = 30, 64, 441
BN_EPS = 1e-5

NQ, D = 8, DIM
R = NQ * HW            # 3528 query-pixel rows per core
M = 126                # rows per chunk
CH = R // M            # 28 chunks
NS = WAY * SHOT        # 25 support maps
HC0 = [(0, 128), (128, 256), (256, 384)]
TAIL = 384
CORE_RANGES = [(0, 8), (8, 16), (15, 23), (22, 30)]


def _split_multi_waits(nc, max_waits=1):
    """walrus CTRL codegen rejects >max_waits sem-waits on one instruction;
    split extras onto preceding drains."""
    for function in nc.m.functions:
        for block in function.blocks:
            new_insts = []
            for inst in block.instructions:
                si = inst.sync_info
                if si is None or si.on_wait is None or len(si.on_wait) <= max_waits:
                    new_insts.append(inst)
                    continue
                waits = list(si.on_wait)
                extra, keep = waits[:-max_waits], waits[-max_waits:]
                ci = 0
                while extra:
                    chunk, extra = extra[:max_waits], extra[max_waits:]
                    new_insts.append(mybir.InstDrain(
                        name=f"{inst.name}-wsplit{ci}", engine=inst.engine,
                        ins=[], outs=[],
                        sync_info=mybir.SyncInfo(on_wait=chunk, on_update=[])))
                    ci += 1
                new_insts.append(copy.replace(
                    inst, sync_info=mybir.SyncInfo(
                        on_wait=keep, on_update=list(si.on_update or []))))
            block.instructions = new_insts


def _build_sel() -> np.ndarray:
    sel = np.zeros((CH, M, NQ), np.float32)
    for c in range(CH):
        for r in range(M):
            sel[c, r, (c * M + r) // HW] = 1.0
    return sel


def _build_kernel() -> bass.Bass:
    nc = bass.Bass("TRN2")
    q_d = nc.dram_tensor("q", [NQ, D, HW], F32, kind="ExternalInput")
    s_d = nc.dram_tensor("s", [NS, D, HW], F32, kind="ExternalInput")
    sel_d = nc.dram_tensor("sel", [CH, M, NQ], F32, kind="ExternalInput")
    feats_d = nc.dram_tensor("feats", [NQ, 3 * WAY], F32, kind="ExternalOutput")

    with TileContext(nc) as tc:
        with tc.tile_pool(name="big", bufs=1) as big:
            # ---------- load ----------
            q_raw = big.tile([D, R], F32)
            s_raw_w = [big.tile([D, SHOT * HW], F32, name=f"sraw{w}", tag=f"sraw{w}")
                       for w in range(WAY)]
            sel_sb = big.tile([M, CH * NQ], F32)
            for w in range(WAY):
                nc.gpsimd.dma_start(
                    s_raw_w[w][:, :].rearrange("d (n h) -> d n h", h=HW),
                    s_d[w * SHOT:(w + 1) * SHOT, :, :].rearrange("n d h -> d n h"))
            nc.gpsimd.dma_start(
                q_raw[:, :].rearrange("d (q h) -> d q h", h=HW),
                q_d[:, :, :].rearrange("q d h -> d q h"))
            nc.gpsimd.dma_start(
                sel_sb[:, :].rearrange("r (c q) -> r c q", q=NQ),
                sel_d[:, :, :].rearrange("c r q -> r c q"))

            # ---------- minimal prologue: stats + fp16 normalize ----------
            sq_scr = big.tile([D, HW], F32)
            q_ss = big.tile([D, NQ], F32)
            s_ss = big.tile([D, NS], F32)
            s_rn = big.tile([D, NS], F32)
            q_rn = big.tile([D, NQ], F32)
            qn = big.tile([D, R], F16)
            sn = big.tile([D, NS * HW], F16)
            # q0 first (pixel chunk 0 needs it), then per-way support stats
            nc.scalar.activation(sq_scr[:, :], q_raw[:, 0:HW],
                                 mybir.ActivationFunctionType.Square,
                                 accum_out=q_ss[:, 0:1])
            nc.scalar.sqrt(q_rn[:, 0:1], q_ss[:, 0:1])
            nc.vector.reciprocal(q_rn[:, 0:1], q_rn[:, 0:1])
            nc.scalar.mul(qn[:, 0:HW], q_raw[:, 0:HW], q_rn[:, 0:1])
            for w in range(WAY):
                for sh in range(SHOT):
                    i = w * SHOT + sh
                    nc.scalar.activation(sq_scr[:, :],
                                         s_raw_w[w][:, sh * HW:(sh + 1) * HW],
                                         mybir.ActivationFunctionType.Square,
                                         accum_out=s_ss[:, i:i + 1])
                nc.scalar.sqrt(s_rn[:, w * SHOT:(w + 1) * SHOT],
                               s_ss[:, w * SHOT:(w + 1) * SHOT])
                nc.vector.reciprocal(s_rn[:, w * SHOT:(w + 1) * SHOT],
                                     s_rn[:, w * SHOT:(w + 1) * SHOT])
                for sh in range(SHOT):
                    i = w * SHOT + sh
                    nc.scalar.mul(sn[:, i * HW:(i + 1) * HW],
                                  s_raw_w[w][:, sh * HW:(sh + 1) * HW],
                                  s_rn[:, i:i + 1])
            for i in range(1, NQ):
                nc.scalar.activation(sq_scr[:, :], q_raw[:, i * HW:(i + 1) * HW],
                                     mybir.ActivationFunctionType.Square,
                                     accum_out=q_ss[:, i:i + 1])
            nc.scalar.sqrt(q_rn[:, 1:], q_ss[:, 1:])
            nc.vector.reciprocal(q_rn[:, 1:], q_rn[:, 1:])
            for i in range(1, NQ):
                nc.scalar.mul(qn[:, i * HW:(i + 1) * HW],
                              q_raw[:, i * HW:(i + 1) * HW], q_rn[:, i:i + 1])

            # ---------- pixel level (dominant; starts as soon as qn/sn land) --
            rs_all = big.tile([M, CH * WAY], F32)
            cand_all = big.tile([M, CH * WAY * 8], F32)
            with tc.tile_pool(name="stg", bufs=STG_BUFS) as stg, \
                 tc.tile_pool(name="psA", bufs=2, space="PSUM") as psA, \
                 tc.tile_pool(name="psB", bufs=1, space="PSUM") as psB:
                for c in range(CH):
                    lhs = qn[:, c * M:(c + 1) * M]
                    for w in range(WAY):
                        g = c * WAY + w
                        A = psA.tile([M, 3 * 512], F32, tag="A")
                        B = psB.tile([M, 2 * 512], F32, tag="B")
                        stage = stg.tile([M, SHOT * HW], STAGE_DT, tag="stage")
                        for sh in range(2):
                            nc.tensor.matmul(
                                B[:, sh * 512:sh * 512 + HW], lhs,
                                sn[:, (w * SHOT + 3 + sh) * HW:(w * SHOT + 4 + sh) * HW],
                                start=True, stop=True)
                        for sh in range(3):
                            nc.tensor.matmul(
                                A[:, sh * 512:sh * 512 + HW], lhs,
                                sn[:, (w * SHOT + sh) * HW:(w * SHOT + sh + 1) * HW],
                                start=True, stop=True)
                        nc.scalar.copy(
                            stage[:, 3 * HW:5 * HW],
                            B[:, :].rearrange("m (b x) -> m b x", x=512)[:, :, :HW])
                        nc.scalar.copy(
                            stage[:, 0:3 * HW],
                            A[:, :].rearrange("m (b x) -> m b x", x=512)[:, :, :HW])
                        nc.vector.max(out=cand_all[:, g * 8:(g + 1) * 8],
                                      in_=stage[:, :])
                nc.vector.reduce_sum(
                    rs_all[:, :],
                    cand_all[:, :].rearrange("m (g k) -> m g k", k=8)[:, :, 0:5],
                    axis=mybir.AxisListType.X)

            # ---------- cosine logits (overlaps pixel tail) ----------
            q_pool = big.tile([D, NQ], F32)
            proto = big.tile([D, WAY], F32)
            mean_scr = big.tile([D, SHOT * HW], F32)
            for i in range(NQ):
                nc.scalar.activation(mean_scr[:, 0:HW], q_raw[:, i * HW:(i + 1) * HW],
                                     mybir.ActivationFunctionType.Identity,
                                     scale=1.0 / HW,
                                     accum_out=q_pool[:, i:i + 1])
            for w in range(WAY):
                nc.scalar.activation(mean_scr[:, :], s_raw_w[w][:, :],
                                     mybir.ActivationFunctionType.Identity,
                                     scale=1.0 / (SHOT * HW),
                                     accum_out=proto[:, w:w + 1])
            psq = big.tile([D, NQ + WAY], F32)
            nc.scalar.square(psq[:, 0:NQ], q_pool[:, :])
            nc.scalar.square(psq[:, NQ:], proto[:, :])
            ones64 = big.tile([D, 1], F32)
            nc.vector.memset(ones64[:, :], 1.0)
            ones8 = big.tile([1, NQ], F32)
            nc.vector.memset(ones8[:, :], 1.0)
            rinv = big.tile([1, NQ + WAY], F32)
            feats = big.tile([NQ, 3 * WAY], F32)
            with tc.tile_pool(name="psnrm", bufs=1, space="PSUM") as psnrm:
                pss = psnrm.tile([1, NQ + WAY], F32)
                nc.tensor.matmul(pss[:, :], ones64[:, :], psq[:, :],
                                 start=True, stop=True)
                nc.scalar.sqrt(rinv[:, :], pss[:, :])
            nc.vector.reciprocal(rinv[:, :], rinv[:, :])
            q_rinv_col = big.tile([NQ, 1], F32)
            nc.gpsimd.dma_start(q_rinv_col[:, :], rinv[0:1, 0:NQ])
            with tc.tile_pool(name="psdot", bufs=1, space="PSUM") as psdot:
                dots = psdot.tile([NQ, WAY], F32)
                nc.tensor.matmul(dots[:, :], q_pool[:, :], proto[:, :],
                                 start=True, stop=True)
                pr_b = psdot.tile([NQ, WAY], F32)
                nc.tensor.matmul(pr_b[:, :], ones8[:, :], rinv[0:1, NQ:],
                                 start=True, stop=True)
                nc.scalar.mul(feats[:, 0:WAY], dots[:, :], q_rinv_col[:, :])
                nc.vector.tensor_mul(feats[:, 0:WAY], feats[:, 0:WAY], pr_b[:, :])

            # ---------- transposes (DMA xbar) for channel level ----------
            q_tail = big.tile([D, NQ * 128], F16)
            s_tail = big.tile([D, NS * 128], F16)
            nc.vector.memset(q_tail[:, :], 0.0)
            nc.vector.memset(s_tail[:, :], 0.0)
            for i in range(NQ):
                nc.scalar.copy(q_tail[:, i * 128:i * 128 + HW - TAIL],
                               qn[:, i * HW + TAIL:(i + 1) * HW])
            for i in range(NS):
                nc.scalar.copy(s_tail[:, i * 128:i * 128 + HW - TAIL],
                               sn[:, i * HW + TAIL:(i + 1) * HW])
            qn_T = big.tile([128, NQ * 4 * D], F16)
            for qi in range(NQ):
                for hc, (h0, h1) in enumerate(HC0):
                    nc.sync.dma_start_transpose(
                        qn_T[0:h1 - h0, (qi * 4 + hc) * D:(qi * 4 + hc + 1) * D],
                        qn[:, qi * HW + h0:qi * HW + h1])
                nc.sync.dma_start_transpose(
                    qn_T[0:128, (qi * 4 + 3) * D:(qi * 4 + 4) * D],
                    q_tail[:, qi * 128:(qi + 1) * 128])
            sl_T = big.tile([128, WAY * 4 * SHOT * D], F16)
            for w in range(WAY):
                for sh in range(SHOT):
                    src0 = (w * SHOT + sh) * HW
                    for hc, (h0, h1) in enumerate(HC0):
                        nc.sync.dma_start_transpose(
                            sl_T[0:h1 - h0,
                                 (w * 4 + hc) * SHOT * D + sh * D:
                                 (w * 4 + hc) * SHOT * D + (sh + 1) * D],
                            sn[:, src0 + h0:src0 + h1])
                    nc.sync.dma_start_transpose(
                        sl_T[0:128,
                             (w * 4 + 3) * SHOT * D + sh * D:
                             (w * 4 + 3) * SHOT * D + (sh + 1) * D],
                        s_tail[:, (w * SHOT + sh) * 128:(w * SHOT + sh + 1) * 128])

            # ---------- channel level (2 queries packed per PSUM tile) -------
            HCN = [128, 128, 128, 128]
            ch_sums = big.tile([128, 4 * WAY], F32)     # part: q (0-63 -> qi, 64-127 -> qi+4)
            cand_ch = big.tile([128, 8], F32)
            half_sel = big.tile([128, 2], F32)
            nc.vector.memset(half_sel[0:D, 0:1], 1.0)
            nc.vector.memset(half_sel[0:D, 1:2], 0.0)
            nc.vector.memset(half_sel[D:, 0:1], 0.0)
            nc.vector.memset(half_sel[D:, 1:2], 1.0)
            with tc.tile_pool(name="psch", bufs=1, space="PSUM") as psch:
                for pair in range(4):                   # qi = pair, qj = pair + 4
                    pch = psch.tile([128, WAY * 512], F32, tag="pch")
                    for half, qi in ((0, pair), (1, pair + 4)):
                        for hc in range(4):
                            hcn = HCN[hc]
                            for w in range(WAY):
                                nc.tensor.matmul(
                                    pch[half * D:half * D + D,
                                        w * 512:w * 512 + SHOT * D],
                                    qn_T[0:hcn, (qi * 4 + hc) * D:(qi * 4 + hc + 1) * D],
                                    sl_T[0:hcn, (w * 4 + hc) * SHOT * D:(w * 4 + hc + 1) * SHOT * D],
                                    start=(hc == 0), stop=(hc == 3))
                    for w in range(WAY):
                        nc.vector.max(out=cand_ch[:, :],
                                      in_=pch[:, w * 512:w * 512 + SHOT * D])
                        nc.vector.reduce_sum(ch_sums[:, pair * WAY + w:pair * WAY + w + 1],
                                             cand_ch[:, 0:5],
                                             axis=mybir.AxisListType.X)
            siml_sb = big.tile([2, 4 * WAY], F32)
            with tc.tile_pool(name="pssl", bufs=1, space="PSUM") as pssl:
                siml = pssl.tile([2, 4 * WAY], F32)
                nc.tensor.matmul(siml[:, :], half_sel[:, :], ch_sums[:, :],
                                 start=True, stop=True)
                nc.scalar.copy(siml_sb[:, :], siml[:, :])
            # [2(half), 4(pair) * 5(w)] -> feats rows q = half*4 + pair
            nc.gpsimd.dma_start(feats[:, WAY:2 * WAY], siml_sb[:, :])

            # ---------- per-query segmented sums ----------
            with tc.tile_pool(name="psP", bufs=1, space="PSUM") as psP:
                simp = psP.tile([NQ, WAY], F32)
                for c in range(CH):
                    nc.tensor.matmul(simp[:, :],
                                     sel_sb[:, c * NQ:(c + 1) * NQ],
                                     rs_all[:, c * WAY:(c + 1) * WAY],
                                     start=(c == 0), stop=(c == CH - 1))
                nc.scalar.copy(feats[:, 2 * WAY:], simp[:, :])

            nc.gpsimd.dma_start(feats_d[:, :], feats[:, :])

    _split_multi_waits(nc, max_waits=1)
    return nc


_NC = None
_SEL = None


def _get_kernel():
    global _NC, _SEL
    if _NC is None:
        _NC = _build_kernel()
        _SEL = _build_sel()
    return _NC, _SEL


def kernel(input1_batch, input2_batch, gamma, beta, conv_w):
    q = np.ascontiguousarray(np.asarray(input1_batch, dtype=np.float32)
                             .reshape(B, Q_NUM, DIM, HW))
    s = np.ascontiguousarray(np.asarray(input2_batch, dtype=np.float32)
                             .reshape(B, WAY * SHOT, DIM, HW))
    gamma = np.asarray(gamma, dtype=np.float32)
    beta = np.asarray(beta, dtype=np.float32)
    w3 = np.asarray(conv_w, dtype=np.float32).reshape(3)

    nc, sel = _get_kernel()
    in_maps = []
    for e in range(B):
        for (lo, hi) in CORE_RANGES:
            in_maps.append({
                "q": np.ascontiguousarray(q[e, lo:hi]),
                "s": s[e],
                "sel": sel,
            })
    res = run_bass_kernel_spmd(nc, in_maps, core_ids=list(range(8)))

    out = np.zeros((B, Q_NUM, WAY), np.float32)
    for e in range(B):
        feats = np.zeros((Q_NUM, 3 * WAY), np.float32)
        for ci, (lo, hi) in enumerate(CORE_RANGES):
            f = res.results[e * 4 + ci]["feats"]
            skip = 1 if ci >= 2 else 0   # drop overlapped duplicate row
            feats[lo + skip:hi] = f[skip:]
        mu = feats.mean(0)
        var = feats.var(0)
        fb = (feats - mu) / np.sqrt(var + BN_EPS) * gamma + beta
        out[e] = w3[0] * fb[:, :WAY] + w3[1] * fb[:, WAY:2 * WAY] + w3[2] * fb[:, 2 * WAY:]
    return out


# revision 10
# speedup vs baseline: 1.4271x; 1.0839x over previous
"""MetaBaseline (retrieval_knn) Trainium2 kernel — 8-core SPMD.

Sharding: each episode's 30 queries are split over 4 cores with ranges
[0:8], [8:16], [15:23], [22:30] (ranges overlap by one query so every core
runs an identical nq=8 program; duplicated rows are dropped on gather).
Each core computes, for its queries, against its episode's full support set:
  - cosine logits (mean-pooled, PE Gram trick for norms)
  - channel-level top-5 similarity (fp16 matmuls + hw top-8 `vector.max`)
  - pixel-level top-5 similarity (dominant cost; fp16 matmuls into PSUM,
    `vector.max` straight from PSUM over 3+2 shot banks, exact hierarchical
    top-5 merge, per-query segmented sums via selector matmuls)
The tiny BatchNorm (batch stats over 30 queries) + dilated conv epilogue runs
on host from the gathered [30, 15] features.

Program order is tuned so the pixel loop (DVE-bound steady state) starts as
early as possible: only the support/query stats + fp16 normalize gate it; the
cosine/channel phases are emitted after it and overlap its tail.
"""
import copy
import numpy as np

import concourse.bass as bass
import concourse.mybir as mybir
from concourse.tile import TileContext
from concourse.bass_utils import run_bass_kernel_spmd

F32 = mybir.dt.float32
F16 = mybir.dt.float16
STAGE_DT = mybir.dt.float32
STG_BUFS = 7

B, WAY, SHOT, K = 2, 5, 5, 5
Q_NUM, DIM, HW = 30, 64, 441
BN_EPS = 1e-5

NQ, D = 8, DIM
R = NQ * HW            # 3528 query-pixel rows per core
M = 126                # rows per chunk
CH = R // M            # 28 chunks
NS = WAY * SHOT        # 25 support maps
HC0 = [(0, 128), (128, 256), (256, 384)]
TAIL = 384
CORE_RANGES = [(0, 8), (8, 16), (15, 23), (22, 30)]


def _split_multi_waits(nc, max_waits=1):
    """walrus CTRL codegen rejects >max_waits sem-waits on one instruction;
    split extras onto preceding drains."""
    for function in nc.m.functions:
        for block in function.blocks:
            new_insts = []
            for inst in block.instructions:
                si = inst.sync_info
                if si is None or si.on_wait is None or len(si.on_wait) <= max_waits:
                    new_insts.append(inst)
                    continue
                waits = list(si.on_wait)
                extra, keep = waits[:-max_waits], waits[-max_waits:]
                ci = 0
                while extra:
                    chunk, extra = extra[:max_waits], extra[max_waits:]
                    new_insts.append(mybir.InstDrain(
                        name=f"{inst.name}-wsplit{ci}", engine=inst.engine,
                        ins=[], outs=[],
                        sync_info=mybir.SyncInfo(on_wait=chunk, on_update=[])))
                    ci += 1
                new_insts.append(copy.replace(
                    inst, sync_info=mybir.SyncInfo(
                        on_wait=keep, on_update=list(si.on_update or []))))
            block.instructions = new_insts


def _build_sel() -> np.ndarray:
    sel = np.zeros((CH, M, NQ), np.float32)
    for c in range(CH):
        for r in range(M):
            sel[c, r, (c * M + r) // HW] = 1.0
    return sel


def _build_kernel() -> bass.Bass:
    nc = bass.Bass("TRN2")
    q_d = nc.dram_tensor("q", [NQ, D, HW], F32, kind="ExternalInput")
    s_d = nc.dram_tensor("s", [NS, D, HW], F32, kind="ExternalInput")
    sel_d = nc.dram_tensor("sel", [CH, M, NQ], F32, kind="ExternalInput")
    feats_d = nc.dram_tensor("feats", [NQ, 3 * WAY], F32, kind="ExternalOutput")

    with TileContext(nc) as tc:
        with tc.tile_pool(name="big", bufs=1) as big:
            # ---------- load ----------
            q_raw = big.tile([D, R], F32)
            s_raw_w = [big.tile([D, SHOT * HW], F32, name=f"sraw{w}", tag=f"sraw{w}")
                       for w in range(WAY)]
            sel_sb = big.tile([M, CH * NQ], F32)
            for w in range(WAY):
                nc.gpsimd.dma_start(
                    s_raw_w[w][:, :].rearrange("d (n h) -> d n h", h=HW),
                    s_d[w * SHOT:(w + 1) * SHOT, :, :].rearrange("n d h -> d n h"))
            nc.gpsimd.dma_start(
                q_raw[:, :].rearrange("d (q h) -> d q h", h=HW),
                q_d[:, :, :].rearrange("q d h -> d q h"))
            nc.gpsimd.dma_start(
                sel_sb[:, :].rearrange("r (c q) -> r c q", q=NQ),
                sel_d[:, :, :].rearrange("c r q -> r c q"))

            # ---------- minimal prologue: stats + fp16 normalize ----------
            sq_scr = big.tile([D, HW], F32)
            q_ss = big.tile([D, NQ], F32)
            s_ss = big.tile([D, NS], F32)
            s_rn = big.tile([D, NS], F32)
            q_rn = big.tile([D, NQ], F32)
            qn = big.tile([D, R], F16)
            sn = big.tile([D, NS * HW], F16)
            # q0 first (pixel chunk 0 needs it), then per-way support stats
            nc.scalar.activation(sq_scr[:, :], q_raw[:, 0:HW],
                                 mybir.ActivationFunctionType.Square,
                                 accum_out=q_ss[:, 0:1])
            nc.scalar.sqrt(q_rn[:, 0:1], q_ss[:, 0:1])
            nc.vector.reciprocal(q_rn[:, 0:1], q_rn[:, 0:1])
            nc.scalar.mul(qn[:, 0:HW], q_raw[:, 0:HW], q_rn[:, 0:1])
            for w in range(WAY):
                for sh in range(SHOT):
                    i = w * SHOT + sh
                    nc.scalar.activation(sq_scr[:, :],
                                         s_raw_w[w][:, sh * HW:(sh + 1) * HW],
                                         mybir.ActivationFunctionType.Square,
                                         accum_out=s_ss[:, i:i + 1])
                nc.scalar.sqrt(s_rn[:, w * SHOT:(w + 1) * SHOT],
                               s_ss[:, w * SHOT:(w + 1) * SHOT])
                nc.vector.reciprocal(s_rn[:, w * SHOT:(w + 1) * SHOT],
                                     s_rn[:, w * SHOT:(w + 1) * SHOT])
                for sh in range(SHOT):
                    i = w * SHOT + sh
                    nc.scalar.mul(sn[:, i * HW:(i + 1) * HW],
                                  s_raw_w[w][:, sh * HW:(sh + 1) * HW],
                                  s_rn[:, i:i + 1])
            for i in range(1, NQ):
                nc.scalar.activation(sq_scr[:, :], q_raw[:, i * HW:(i + 1) * HW],
                                     mybir.ActivationFunctionType.Square,
                                     accum_out=q_ss[:, i:i + 1])
            nc.scalar.sqrt(q_rn[:, 1:], q_ss[:, 1:])
            nc.vector.reciprocal(q_rn[:, 1:], q_rn[:, 1:])
            for i in range(1, NQ):
                nc.scalar.mul(qn[:, i * HW:(i + 1) * HW],
                              q_raw[:, i * HW:(i + 1) * HW], q_rn[:, i:i + 1])

            # ---------- pixel level (dominant; starts as soon as qn/sn land) --
            rs_all = big.tile([M, CH * WAY], F32)
            cand_all = big.tile([M, CH * WAY * 8], F32)
            with tc.tile_pool(name="stg", bufs=STG_BUFS) as stg, \
                 tc.tile_pool(name="psA", bufs=2, space="PSUM") as psA, \
                 tc.tile_pool(name="psB", bufs=1, space="PSUM") as psB:
                for c in range(CH):
                    lhs = qn[:, c * M:(c + 1) * M]
                    for w in range(WAY):
                        g = c * WAY + w
                        A = psA.tile([M, 3 * 512], F32, tag="A")
                        B = psB.tile([M, 2 * 512], F32, tag="B")
                        stage = stg.tile([M, SHOT * HW], STAGE_DT, tag="stage")
                        for sh in range(2):
                            nc.tensor.matmul(
                                B[:, sh * 512:sh * 512 + HW], lhs,
                                sn[:, (w * SHOT + 3 + sh) * HW:(w * SHOT + 4 + sh) * HW],
                                start=True, stop=True)
                        for sh in range(3):
                            nc.tensor.matmul(
                                A[:, sh * 512:sh * 512 + HW], lhs,
                                sn[:, (w * SHOT + sh) * HW:(w * SHOT + sh + 1) * HW],
                                start=True, stop=True)
                        nc.scalar.copy(
                            stage[:, 3 * HW:5 * HW],
                            B[:, :].rearrange("m (b x) -> m b x", x=512)[:, :, :HW])
                        nc.scalar.copy(
                            stage[:, 0:3 * HW],
                            A[:, :].rearrange("m (b x) -> m b x", x=512)[:, :, :HW])
                        nc.vector.max(out=cand_all[:, g * 8:(g + 1) * 8],
                                      in_=stage[:, :])
                nc.vector.reduce_sum(
                    rs_all[:, :],
                    cand_all[:, :].rearrange("m (g k) -> m g k", k=8)[:, :, 0:5],
                    axis=mybir.AxisListType.X)

            # ---------- cosine logits (overlaps pixel tail) ----------
            q_pool = big.tile([D, NQ], F32)
            proto = big.tile([D, WAY], F32)
            mean_scr = big.tile([D, SHOT * HW], F32)
            for i in range(NQ):
                nc.scalar.activation(mean_scr[:, 0:HW], q_raw[:, i * HW:(i + 1) * HW],
                                     mybir.ActivationFunctionType.Identity,
                                     scale=1.0 / HW,
                                     accum_out=q_pool[:, i:i + 1])
            for w in range(WAY):
                nc.scalar.activation(mean_scr[:, :], s_raw_w[w][:, :],
                                     mybir.ActivationFunctionType.Identity,
                                     scale=1.0 / (SHOT * HW),
                                     accum_out=proto[:, w:w + 1])
            psq = big.tile([D, NQ + WAY], F32)
            nc.scalar.square(psq[:, 0:NQ], q_pool[:, :])
            nc.scalar.square(psq[:, NQ:], proto[:, :])
            ones64 = big.tile([D, 1], F32)
            nc.vector.memset(ones64[:, :], 1.0)
            ones8 = big.tile([1, NQ], F32)
            nc.vector.memset(ones8[:, :], 1.0)
            rinv = big.tile([1, NQ + WAY], F32)
            feats = big.tile([NQ, 3 * WAY], F32)
            with tc.tile_pool(name="psnrm", bufs=1, space="PSUM") as psnrm:
                pss = psnrm.tile([1, NQ + WAY], F32)
                nc.tensor.matmul(pss[:, :], ones64[:, :], psq[:, :],
                                 start=True, stop=True)
                nc.scalar.sqrt(rinv[:, :], pss[:, :])
            nc.vector.reciprocal(rinv[:, :], rinv[:, :])
            q_rinv_col = big.tile([NQ, 1], F32)
            nc.gpsimd.dma_start(q_rinv_col[:, :], rinv[0:1, 0:NQ])
            with tc.tile_pool(name="psdot", bufs=1, space="PSUM") as psdot:
                dots = psdot.tile([NQ, WAY], F32)
                nc.tensor.matmul(dots[:, :], q_pool[:, :], proto[:, :],
                                 start=True, stop=True)
                pr_b = psdot.tile([NQ, WAY], F32)
                nc.tensor.matmul(pr_b[:, :], ones8[:, :], rinv[0:1, NQ:],
                                 start=True, stop=True)
                nc.scalar.mul(feats[:, 0:WAY], dots[:, :], q_rinv_col[:, :])
                nc.vector.tensor_mul(feats[:, 0:WAY], feats[:, 0:WAY], pr_b[:, :])

            # ---------- transposes (DMA xbar) for channel level ----------
            q_tail = big.tile([D, NQ * 128], F16)
            s_tail = big.tile([D, NS * 128], F16)
            nc.vector.memset(q_tail[:, :], 0.0)
            nc.vector.memset(s_tail[:, :], 0.0)
            for i in range(NQ):
                nc.scalar.copy(q_tail[:, i * 128:i * 128 + HW - TAIL],
                               qn[:, i * HW + TAIL:(i + 1) * HW])
            for i in range(NS):
                nc.scalar.copy(s_tail[:, i * 128:i * 128 + HW - TAIL],
                               sn[:, i * HW + TAIL:(i + 1) * HW])
            qn_T = big.tile([128, NQ * 4 * D], F16)
            for qi in range(NQ):
                for hc, (h0, h1) in enumerate(HC0):
                    nc.sync.dma_start_transpose(
                        qn_T[0:h1 - h0, (qi * 4 + hc) * D:(qi * 4 + hc + 1) * D],
                        qn[:, qi * HW + h0:qi * HW + h1])
                nc.sync.dma_start_transpose(
                    qn_T[0:128, (qi * 4 + 3) * D:(qi * 4 + 4) * D],
                    q_tail[:, qi * 128:(qi + 1) * 128])
            sl_T = big.tile([128, WAY * 4 * SHOT * D], F16)
            for w in range(WAY):
                for sh in range(SHOT):
                    src0 = (w * SHOT + sh) * HW
                    for hc, (h0, h1) in enumerate(HC0):
                        nc.sync.dma_start_transpose(
                            sl_T[0:h1 - h0,
                                 (w * 4 + hc) * SHOT * D + sh * D:
                                 (w * 4 + hc) * SHOT * D + (sh + 1) * D],
                            sn[:, src0 + h0:src0 + h1])
                    nc.sync.dma_start_transpose(
                        sl_T[0:128,
                             (w * 4 + 3) * SHOT * D + sh * D:
                             (w * 4 + 3) * SHOT * D + (sh + 1) * D],
                        s_tail[:, (w * SHOT + sh) * 128:(w * SHOT + sh + 1) * 128])

            # ---------- channel level (2 queries packed per PSUM tile) -------
            HCN = [128, 128, 128, 128]
            ch_sums = big.tile([128, 4 * WAY], F32)     # part: q (0-63 -> qi, 64-127 -> qi+4)
            cand_ch = big.tile([128, 8], F32)
            half_sel = big.tile([128, 2], F32)
            nc.vector.memset(half_sel[0:D, 0:1], 1.0)
            nc.vector.memset(half_sel[0:D, 1:2], 0.0)
            nc.vector.memset(half_sel[D:, 0:1], 0.0)
            nc.vector.memset(half_sel[D:, 1:2], 1.0)
            with tc.tile_pool(name="psch", bufs=8, space="PSUM") as psch:
                for pair in range(4):                   # qi = pair, qj = pair + 4
                    for w in range(WAY):
                        pch = psch.tile([128, 512], F32, tag="pch")
                        for half, qi in ((0, pair), (1, pair + 4)):
                            for hc in range(4):
                                hcn = HCN[hc]
                                nc.tensor.matmul(
                                    pch[half * D:half * D + D, 0:SHOT * D],
                                    qn_T[0:hcn, (qi * 4 + hc) * D:(qi * 4 + hc + 1) * D],
                                    sl_T[0:hcn, (w * 4 + hc) * SHOT * D:(w * 4 + hc + 1) * SHOT * D],
                                    start=(hc == 0), stop=(hc == 3))
                        nc.vector.max(out=cand_ch[:, :], in_=pch[:, 0:SHOT * D])
                        nc.vector.reduce_sum(ch_sums[:, pair * WAY + w:pair * WAY + w + 1],
                                             cand_ch[:, 0:5],
                                             axis=mybir.AxisListType.X)
            siml_sb = big.tile([2, 4 * WAY], F32)
            with tc.tile_pool(name="pssl", bufs=1, space="PSUM") as pssl:
                siml = pssl.tile([2, 4 * WAY], F32)
                nc.tensor.matmul(siml[:, :], half_sel[:, :], ch_sums[:, :],
                                 start=True, stop=True)
                nc.scalar.copy(siml_sb[:, :], siml[:, :])
            # [2(half), 4(pair) * 5(w)] -> feats rows q = half*4 + pair
            nc.gpsimd.dma_start(feats[:, WAY:2 * WAY], siml_sb[:, :])

            # ---------- per-query segmented sums ----------
            with tc.tile_pool(name="psP", bufs=1, space="PSUM") as psP:
                simp = psP.tile([NQ, WAY], F32)
                for c in range(CH):
                    nc.tensor.matmul(simp[:, :],
                                     sel_sb[:, c * NQ:(c + 1) * NQ],
                                     rs_all[:, c * WAY:(c + 1) * WAY],
                                     start=(c == 0), stop=(c == CH - 1))
                nc.scalar.copy(feats[:, 2 * WAY:], simp[:, :])

            nc.gpsimd.dma_start(feats_d[:, :], feats[:, :])

    _split_multi_waits(nc, max_waits=1)
    return nc


_NC = None
_SEL = None


def _get_kernel():
    global _NC, _SEL
    if _NC is None:
        _NC = _build_kernel()
        _SEL = _build_sel()
    return _NC, _SEL


def kernel(input1_batch, input2_batch, gamma, beta, conv_w):
    q = np.ascontiguousarray(np.asarray(input1_batch, dtype=np.float32)
                             .reshape(B, Q_NUM, DIM, HW))
    s = np.ascontiguousarray(np.asarray(input2_batch, dtype=np.float32)
                             .reshape(B, WAY * SHOT, DIM, HW))
    gamma = np.asarray(gamma, dtype=np.float32)
    beta = np.asarray(beta, dtype=np.float32)
    w3 = np.asarray(conv_w, dtype=np.float32).reshape(3)

    nc, sel = _get_kernel()
    in_maps = []
    for e in range(B):
        for (lo, hi) in CORE_RANGES:
            in_maps.append({
                "q": np.ascontiguousarray(q[e, lo:hi]),
                "s": s[e],
                "sel": sel,
            })
    res = run_bass_kernel_spmd(nc, in_maps, core_ids=list(range(8)))

    out = np.zeros((B, Q_NUM, WAY), np.float32)
    for e in range(B):
        feats = np.zeros((Q_NUM, 3 * WAY), np.float32)
        for ci, (lo, hi) in enumerate(CORE_RANGES):
            f = res.results[e * 4 + ci]["feats"]
            skip = 1 if ci >= 2 else 0   # drop overlapped duplicate row
            feats[lo + skip:hi] = f[skip:]
        mu = feats.mean(0)
        var = feats.var(0)
        fb = (feats - mu) / np.sqrt(var + BN_EPS) * gamma + beta
        out[e] = w3[0] * fb[:, :WAY] + w3[1] * fb[:, WAY:2 * WAY] + w3[2] * fb[:, 2 * WAY:]
    return out


# revision 11
# speedup vs baseline: 1.4745x; 1.0332x over previous
"""MetaBaseline (retrieval_knn) Trainium2 kernel — 8-core SPMD.

Sharding: each episode's 30 queries are split over 4 cores with ranges
[0:8], [8:16], [15:23], [22:30] (ranges overlap by one query so every core
runs an identical nq=8 program; duplicated rows are dropped on gather).
Each core computes, for its queries, against its episode's full support set:
  - cosine logits (mean-pooled, PE Gram trick for norms)
  - channel-level top-5 similarity (fp16 matmuls + hw top-8 `vector.max`)
  - pixel-level top-5 similarity (dominant cost; fp16 matmuls into PSUM,
    `vector.max` straight from PSUM over 3+2 shot banks, exact hierarchical
    top-5 merge, per-query segmented sums via selector matmuls)
The tiny BatchNorm (batch stats over 30 queries) + dilated conv epilogue runs
on host from the gathered [30, 15] features.

Program order is tuned so the pixel loop (DVE-bound steady state) starts as
early as possible: only the support/query stats + fp16 normalize gate it; the
cosine/channel phases are emitted after it and overlap its tail.
"""
import copy
import numpy as np

import concourse.bass as bass
import concourse.mybir as mybir
from concourse.tile import TileContext
from concourse.bass_utils import run_bass_kernel_spmd

F32 = mybir.dt.float32
F16 = mybir.dt.float16
STAGE_DT = mybir.dt.float32
STG_BUFS = 7

B, WAY, SHOT, K = 2, 5, 5, 5
Q_NUM, DIM, HW = 30, 64, 441
BN_EPS = 1e-5

NQ, D = 8, DIM
R = NQ * HW            # 3528 query-pixel rows per core
M = 126                # rows per chunk
CH = R // M            # 28 chunks
NS = WAY * SHOT        # 25 support maps
HC0 = [(0, 128), (128, 256), (256, 384)]
TAIL = 384
CORE_RANGES = [(0, 8), (8, 16), (15, 23), (22, 30)]


def _split_multi_waits(nc, max_waits=1):
    """walrus CTRL codegen rejects >max_waits sem-waits on one instruction;
    split extras onto preceding drains."""
    for function in nc.m.functions:
        for block in function.blocks:
            new_insts = []
            for inst in block.instructions:
                si = inst.sync_info
                if si is None or si.on_wait is None or len(si.on_wait) <= max_waits:
                    new_insts.append(inst)
                    continue
                waits = list(si.on_wait)
                extra, keep = waits[:-max_waits], waits[-max_waits:]
                ci = 0
                while extra:
                    chunk, extra = extra[:max_waits], extra[max_waits:]
                    new_insts.append(mybir.InstDrain(
                        name=f"{inst.name}-wsplit{ci}", engine=inst.engine,
                        ins=[], outs=[],
                        sync_info=mybir.SyncInfo(on_wait=chunk, on_update=[])))
                    ci += 1
                new_insts.append(copy.replace(
                    inst, sync_info=mybir.SyncInfo(
                        on_wait=keep, on_update=list(si.on_update or []))))
            block.instructions = new_insts


def _build_sel() -> np.ndarray:
    sel = np.zeros((CH, M, NQ), np.float32)
    for c in range(CH):
        for r in range(M):
            sel[c, r, (c * M + r) // HW] = 1.0
    return sel


def _build_kernel() -> bass.Bass:
    nc = bass.Bass("TRN2")
    q_d = nc.dram_tensor("q", [NQ, D, HW], F32, kind="ExternalInput")
    s_d = nc.dram_tensor("s", [NS, D, HW], F32, kind="ExternalInput")
    sel_d = nc.dram_tensor("sel", [CH, M, NQ], F32, kind="ExternalInput")
    feats_d = nc.dram_tensor("feats", [NQ, 3 * WAY], F32, kind="ExternalOutput")

    with TileContext(nc) as tc:
        with tc.tile_pool(name="big", bufs=1) as big:
            # ---------- load ----------
            q_raw = big.tile([D, R], F32)
            s_raw_w = [big.tile([D, SHOT * HW], F32, name=f"sraw{w}", tag=f"sraw{w}")
                       for w in range(WAY)]
            sel_sb = big.tile([M, CH * NQ], F32)
            for w in range(WAY):
                nc.gpsimd.dma_start(
                    s_raw_w[w][:, :].rearrange("d (n h) -> d n h", h=HW),
                    s_d[w * SHOT:(w + 1) * SHOT, :, :].rearrange("n d h -> d n h"))
            nc.gpsimd.dma_start(q_raw[:, 0:HW], q_d[0, :, :])
            nc.gpsimd.dma_start(
                q_raw[:, HW:].rearrange("d (q h) -> d q h", h=HW),
                q_d[1:, :, :].rearrange("q d h -> d q h"))
            nc.gpsimd.dma_start(
                sel_sb[:, :].rearrange("r (c q) -> r c q", q=NQ),
                sel_d[:, :, :].rearrange("c r q -> r c q"))

            # ---------- minimal prologue: stats + fp16 normalize ----------
            sq_scr = big.tile([D, HW], F32)
            q_ss = big.tile([D, NQ], F32)
            s_ss = big.tile([D, NS], F32)
            s_rn = big.tile([D, NS], F32)
            q_rn = big.tile([D, NQ], F32)
            qn = big.tile([D, R], F16)
            sn = big.tile([D, NS * HW], F16)
            # q0 first (pixel chunk 0 needs it), then per-way support stats
            nc.scalar.activation(sq_scr[:, :], q_raw[:, 0:HW],
                                 mybir.ActivationFunctionType.Square,
                                 accum_out=q_ss[:, 0:1])
            nc.scalar.sqrt(q_rn[:, 0:1], q_ss[:, 0:1])
            nc.vector.reciprocal(q_rn[:, 0:1], q_rn[:, 0:1])
            nc.scalar.mul(qn[:, 0:HW], q_raw[:, 0:HW], q_rn[:, 0:1])
            for w in range(WAY):
                for sh in range(SHOT):
                    i = w * SHOT + sh
                    nc.scalar.activation(sq_scr[:, :],
                                         s_raw_w[w][:, sh * HW:(sh + 1) * HW],
                                         mybir.ActivationFunctionType.Square,
                                         accum_out=s_ss[:, i:i + 1])
                nc.scalar.sqrt(s_rn[:, w * SHOT:(w + 1) * SHOT],
                               s_ss[:, w * SHOT:(w + 1) * SHOT])
                nc.vector.reciprocal(s_rn[:, w * SHOT:(w + 1) * SHOT],
                                     s_rn[:, w * SHOT:(w + 1) * SHOT])
                for sh in range(SHOT):
                    i = w * SHOT + sh
                    nc.vector.tensor_scalar_mul(sn[:, i * HW:(i + 1) * HW],
                                                s_raw_w[w][:, sh * HW:(sh + 1) * HW],
                                                s_rn[:, i:i + 1])
            for i in range(1, NQ):
                nc.scalar.activation(sq_scr[:, :], q_raw[:, i * HW:(i + 1) * HW],
                                     mybir.ActivationFunctionType.Square,
                                     accum_out=q_ss[:, i:i + 1])
            nc.scalar.sqrt(q_rn[:, 1:], q_ss[:, 1:])
            nc.vector.reciprocal(q_rn[:, 1:], q_rn[:, 1:])
            for i in range(1, NQ):
                nc.scalar.mul(qn[:, i * HW:(i + 1) * HW],
                              q_raw[:, i * HW:(i + 1) * HW], q_rn[:, i:i + 1])

            # ---------- pixel level (dominant; starts as soon as qn/sn land) --
            rs_all = big.tile([M, CH * WAY], F32)
            cand_all = big.tile([M, CH * WAY * 8], F32)
            with tc.tile_pool(name="stg", bufs=STG_BUFS) as stg, \
                 tc.tile_pool(name="psA", bufs=2, space="PSUM") as psA, \
                 tc.tile_pool(name="psB", bufs=1, space="PSUM") as psB:
                for c in range(CH):
                    lhs = qn[:, c * M:(c + 1) * M]
                    for w in range(WAY):
                        g = c * WAY + w
                        A = psA.tile([M, 3 * 512], F32, tag="A")
                        B = psB.tile([M, 2 * 512], F32, tag="B")
                        stage = stg.tile([M, SHOT * HW], STAGE_DT, tag="stage")
                        for sh in range(2):
                            nc.tensor.matmul(
                                B[:, sh * 512:sh * 512 + HW], lhs,
                                sn[:, (w * SHOT + 3 + sh) * HW:(w * SHOT + 4 + sh) * HW],
                                start=True, stop=True)
                        for sh in range(3):
                            nc.tensor.matmul(
                                A[:, sh * 512:sh * 512 + HW], lhs,
                                sn[:, (w * SHOT + sh) * HW:(w * SHOT + sh + 1) * HW],
                                start=True, stop=True)
                        nc.scalar.copy(
                            stage[:, 3 * HW:5 * HW],
                            B[:, :].rearrange("m (b x) -> m b x", x=512)[:, :, :HW])
                        nc.scalar.copy(
                            stage[:, 0:3 * HW],
                            A[:, :].rearrange("m (b x) -> m b x", x=512)[:, :, :HW])
                        nc.vector.max(out=cand_all[:, g * 8:(g + 1) * 8],
                                      in_=stage[:, :])
                nc.vector.reduce_sum(
                    rs_all[:, :],
                    cand_all[:, :].rearrange("m (g k) -> m g k", k=8)[:, :, 0:5],
                    axis=mybir.AxisListType.X)

            # ---------- cosine logits (overlaps pixel tail) ----------
            q_pool = big.tile([D, NQ], F32)
            proto = big.tile([D, WAY], F32)
            mean_scr = big.tile([D, SHOT * HW], F32)
            for i in range(NQ):
                nc.scalar.activation(mean_scr[:, 0:HW], q_raw[:, i * HW:(i + 1) * HW],
                                     mybir.ActivationFunctionType.Identity,
                                     scale=1.0 / HW,
                                     accum_out=q_pool[:, i:i + 1])
            for w in range(WAY):
                nc.scalar.activation(mean_scr[:, :], s_raw_w[w][:, :],
                                     mybir.ActivationFunctionType.Identity,
                                     scale=1.0 / (SHOT * HW),
                                     accum_out=proto[:, w:w + 1])
            psq = big.tile([D, NQ + WAY], F32)
            nc.scalar.square(psq[:, 0:NQ], q_pool[:, :])
            nc.scalar.square(psq[:, NQ:], proto[:, :])
            ones64 = big.tile([D, 1], F32)
            nc.vector.memset(ones64[:, :], 1.0)
            ones8 = big.tile([1, NQ], F32)
            nc.vector.memset(ones8[:, :], 1.0)
            rinv = big.tile([1, NQ + WAY], F32)
            feats = big.tile([NQ, 3 * WAY], F32)
            with tc.tile_pool(name="psnrm", bufs=1, space="PSUM") as psnrm:
                pss = psnrm.tile([1, NQ + WAY], F32)
                nc.tensor.matmul(pss[:, :], ones64[:, :], psq[:, :],
                                 start=True, stop=True)
                nc.scalar.sqrt(rinv[:, :], pss[:, :])
            nc.vector.reciprocal(rinv[:, :], rinv[:, :])
            q_rinv_col = big.tile([NQ, 1], F32)
            nc.gpsimd.dma_start(q_rinv_col[:, :], rinv[0:1, 0:NQ])
            with tc.tile_pool(name="psdot", bufs=1, space="PSUM") as psdot:
                dots = psdot.tile([NQ, WAY], F32)
                nc.tensor.matmul(dots[:, :], q_pool[:, :], proto[:, :],
                                 start=True, stop=True)
                pr_b = psdot.tile([NQ, WAY], F32)
                nc.tensor.matmul(pr_b[:, :], ones8[:, :], rinv[0:1, NQ:],
                                 start=True, stop=True)
                nc.scalar.mul(feats[:, 0:WAY], dots[:, :], q_rinv_col[:, :])
                nc.vector.tensor_mul(feats[:, 0:WAY], feats[:, 0:WAY], pr_b[:, :])

            # ---------- transposes (DMA xbar) for channel level ----------
            q_tail = big.tile([D, NQ * 128], F16)
            s_tail = big.tile([D, NS * 128], F16)
            nc.vector.memset(q_tail[:, :], 0.0)
            nc.vector.memset(s_tail[:, :], 0.0)
            for i in range(NQ):
                nc.scalar.copy(q_tail[:, i * 128:i * 128 + HW - TAIL],
                               qn[:, i * HW + TAIL:(i + 1) * HW])
            for i in range(NS):
                nc.scalar.copy(s_tail[:, i * 128:i * 128 + HW - TAIL],
                               sn[:, i * HW + TAIL:(i + 1) * HW])
            qn_T = big.tile([128, NQ * 4 * D], F16)
            for qi in range(NQ):
                for hc, (h0, h1) in enumerate(HC0):
                    nc.sync.dma_start_transpose(
                        qn_T[0:h1 - h0, (qi * 4 + hc) * D:(qi * 4 + hc + 1) * D],
                        qn[:, qi * HW + h0:qi * HW + h1])
                nc.sync.dma_start_transpose(
                    qn_T[0:128, (qi * 4 + 3) * D:(qi * 4 + 4) * D],
                    q_tail[:, qi * 128:(qi + 1) * 128])
            sl_T = big.tile([128, WAY * 4 * SHOT * D], F16)
            for w in range(WAY):
                for sh in range(SHOT):
                    src0 = (w * SHOT + sh) * HW
                    for hc, (h0, h1) in enumerate(HC0):
                        nc.sync.dma_start_transpose(
                            sl_T[0:h1 - h0,
                                 (w * 4 + hc) * SHOT * D + sh * D:
                                 (w * 4 + hc) * SHOT * D + (sh + 1) * D],
                            sn[:, src0 + h0:src0 + h1])
                    nc.sync.dma_start_transpose(
                        sl_T[0:128,
                             (w * 4 + 3) * SHOT * D + sh * D:
                             (w * 4 + 3) * SHOT * D + (sh + 1) * D],
                        s_tail[:, (w * SHOT + sh) * 128:(w * SHOT + sh + 1) * 128])

            # ---------- channel level (2 queries packed per PSUM tile) -------
            HCN = [128, 128, 128, 128]
            ch_sums = big.tile([128, 4 * WAY], F32)     # part: q (0-63 -> qi, 64-127 -> qi+4)
            cand_ch = big.tile([128, 8], F32)
            half_sel = big.tile([128, 2], F32)
            nc.vector.memset(half_sel[0:D, 0:1], 1.0)
            nc.vector.memset(half_sel[0:D, 1:2], 0.0)
            nc.vector.memset(half_sel[D:, 0:1], 0.0)
            nc.vector.memset(half_sel[D:, 1:2], 1.0)
            with tc.tile_pool(name="psch", bufs=8, space="PSUM") as psch:
                for pair in range(4):                   # qi = pair, qj = pair + 4
                    for w in range(WAY):
                        pch = psch.tile([128, 512], F32, tag="pch")
                        for half, qi in ((0, pair), (1, pair + 4)):
                            for hc in range(4):
                                hcn = HCN[hc]
                                nc.tensor.matmul(
                                    pch[half * D:half * D + D, 0:SHOT * D],
                                    qn_T[0:hcn, (qi * 4 + hc) * D:(qi * 4 + hc + 1) * D],
                                    sl_T[0:hcn, (w * 4 + hc) * SHOT * D:(w * 4 + hc + 1) * SHOT * D],
                                    start=(hc == 0), stop=(hc == 3))
                        nc.vector.max(out=cand_ch[:, :], in_=pch[:, 0:SHOT * D])
                        nc.vector.reduce_sum(ch_sums[:, pair * WAY + w:pair * WAY + w + 1],
                                             cand_ch[:, 0:5],
                                             axis=mybir.AxisListType.X)
            siml_sb = big.tile([2, 4 * WAY], F32)
            with tc.tile_pool(name="pssl", bufs=1, space="PSUM") as pssl:
                siml = pssl.tile([2, 4 * WAY], F32)
                nc.tensor.matmul(siml[:, :], half_sel[:, :], ch_sums[:, :],
                                 start=True, stop=True)
                nc.scalar.copy(siml_sb[:, :], siml[:, :])
            # [2(half), 4(pair) * 5(w)] -> feats rows q = half*4 + pair
            nc.gpsimd.dma_start(feats[:, WAY:2 * WAY], siml_sb[:, :])

            # ---------- per-query segmented sums ----------
            with tc.tile_pool(name="psP", bufs=1, space="PSUM") as psP:
                simp = psP.tile([NQ, WAY], F32)
                for c in range(CH):
                    nc.tensor.matmul(simp[:, :],
                                     sel_sb[:, c * NQ:(c + 1) * NQ],
                                     rs_all[:, c * WAY:(c + 1) * WAY],
                                     start=(c == 0), stop=(c == CH - 1))
                nc.scalar.copy(feats[:, 2 * WAY:], simp[:, :])

            nc.gpsimd.dma_start(feats_d[:, :], feats[:, :])

    _split_multi_waits(nc, max_waits=1)
    return nc


_NC = None
_SEL = None


def _get_kernel():
    global _NC, _SEL
    if _NC is None:
        _NC = _build_kernel()
        _SEL = _build_sel()
    return _NC, _SEL


def kernel(input1_batch, input2_batch, gamma, beta, conv_w):
    q = np.ascontiguousarray(np.asarray(input1_batch, dtype=np.float32)
                             .reshape(B, Q_NUM, DIM, HW))
    s = np.ascontiguousarray(np.asarray(input2_batch, dtype=np.float32)
                             .reshape(B, WAY * SHOT, DIM, HW))
    gamma = np.asarray(gamma, dtype=np.float32)
    beta = np.asarray(beta, dtype=np.float32)
    w3 = np.asarray(conv_w, dtype=np.float32).reshape(3)

    nc, sel = _get_kernel()
    in_maps = []
    for e in range(B):
        for (lo, hi) in CORE_RANGES:
            in_maps.append({
                "q": np.ascontiguousarray(q[e, lo:hi]),
                "s": s[e],
                "sel": sel,
            })
    res = run_bass_kernel_spmd(nc, in_maps, core_ids=list(range(8)))

    out = np.zeros((B, Q_NUM, WAY), np.float32)
    for e in range(B):
        feats = np.zeros((Q_NUM, 3 * WAY), np.float32)
        for ci, (lo, hi) in enumerate(CORE_RANGES):
            f = res.results[e * 4 + ci]["feats"]
            skip = 1 if ci >= 2 else 0   # drop overlapped duplicate row
            feats[lo + skip:hi] = f[skip:]
        mu = feats.mean(0)
        var = feats.var(0)
        fb = (feats - mu) / np.sqrt(var + BN_EPS) * gamma + beta
        out[e] = w3[0] * fb[:, :WAY] + w3[1] * fb[:, WAY:2 * WAY] + w3[2] * fb[:, 2 * WAY:]
    return out
